# revision 1
# baseline (speedup 1.0000x reference)
"""Trainium2 Bass kernel for nn_GatedShortConvBlock.

Full inputs -> full output. Data-parallel over batch: batch b -> core b.
Per-core dataflow (T=4096 tokens, D=1024, H=2048, K=4):
  LN stats on DVE (bn_stats over natural-layout x tiles); normalization
  applied in [d, t] layout to a host-pre-transposed xT via PE rank-1
  broadcast of the mean/var rows -> in-proj f32r matmuls (N=512) ->
  sigmoid gates on ACT (+bias fused) -> depthwise causal conv on DVE
  via scalar_tensor_tensor FMAs ([h, t] layout, per-partition weights)
  -> out-proj f32r matmuls -> +bias +residual -> out.
Weights are pre-transposed/reordered on the host so every DMA is a
large contiguous transfer and every matmul operand is layout-native.
"""

import sys
import types

import numpy as np

B, T, D, H = 8, 4096, 1024, 2048
KCONV = 4
N_CORES = 8
TB = 512            # tokens per t-block
NTB = T // TB       # 8
NSUB = TB // 128    # 4
KC = H // 128       # 16 channel blocks
DC = D // 128       # 8 d-chunks
EPS = 1e-5

_CACHE = {}


def _install_ntff_hook():
    if "antenv.axon_hooks" in sys.modules:
        return
    try:
        import trn_agent_boot.trn_boot as tb

        hook = tb._ntff_profile_via_ctypes("/opt/axon/libaxon_pjrt.so")
        mod = types.ModuleType("antenv.axon_hooks")
        mod.get_axon_ntff_profile_hook = lambda: hook
        mod.set_axon_ntff_profile_hook = lambda h: None
        sys.modules["antenv.axon_hooks"] = mod
    except Exception:
        pass


def _build_nc():
    import concourse.bass as bass  # noqa: F401
    import concourse.tile as tile
    from concourse import bacc, mybir

    f32 = mybir.dt.float32
    f32r = mybir.dt.float32r
    AF = mybir.ActivationFunctionType
    OP = mybir.AluOpType

    nc = bacc.Bacc("TRN2", target_bir_lowering=False, debug=False,
                   num_devices=N_CORES)
    x_d = nc.dram_tensor("x", [T, D], f32, kind="ExternalInput").ap()
    xt_d = nc.dram_tensor("xt_proc", [NTB, 128, DC, TB], f32,
                          kind="ExternalInput").ap()
    win_d = nc.dram_tensor("w_in_proc", [KC, 128, 3 * DC * 128], f32,
                           kind="ExternalInput").ap()
    d_d = nc.dram_tensor("d_proc", [128, KC, 3], f32,
                         kind="ExternalInput").ap()
    cw_d = nc.dram_tensor("cw_proc", [128, KC, KCONV], f32,
                          kind="ExternalInput").ap()
    cb_d = nc.dram_tensor("cb_proc", [128, KC], f32,
                          kind="ExternalInput").ap()
    wout_d = nc.dram_tensor("w_out_proc", [128, KC, D], f32,
                            kind="ExternalInput").ap()
    bout_d = nc.dram_tensor("bout_rep", [128, D], f32,
                            kind="ExternalInput").ap()
    iden_d = nc.dram_tensor("iden", [128, 128], f32,
                            kind="ExternalInput").ap()
    out_d = nc.dram_tensor("out", [T, D], f32, kind="ExternalOutput").ap()

    from contextlib import ExitStack

    with tile.TileContext(nc) as tc, ExitStack() as ctx:
        consts = ctx.enter_context(tc.tile_pool(name="consts", bufs=1))
        wgp = ctx.enter_context(tc.tile_pool(name="wg", bufs=2))
        xp = ctx.enter_context(tc.tile_pool(name="xp", bufs=2))
        lnp = ctx.enter_context(tc.tile_pool(name="ln", bufs=4))
        xtp = ctx.enter_context(tc.tile_pool(name="xtp", bufs=2))
        rowp = ctx.enter_context(tc.tile_pool(name="rowp", bufs=1))
        scp = ctx.enter_context(tc.tile_pool(name="scp", bufs=2))
        rsp = ctx.enter_context(tc.tile_pool(name="rsp", bufs=1))
        hid2p = ctx.enter_context(tc.tile_pool(name="hid2", bufs=1))
        sgp = ctx.enter_context(tc.tile_pool(name="sg", bufs=2))
        hbp = ctx.enter_context(tc.tile_pool(name="hb", bufs=2))
        accp = ctx.enter_context(tc.tile_pool(name="acc", bufs=2))
        outp = ctx.enter_context(tc.tile_pool(name="outp", bufs=2))
        xrp = ctx.enter_context(tc.tile_pool(name="xr", bufs=2))
        pst = ctx.enter_context(tc.tile_pool(name="pst", bufs=3, space="PSUM"))
        psm = ctx.enter_context(tc.tile_pool(name="psm", bufs=5, space="PSUM"))
        if True:
            wout_sb = consts.tile([128, KC, D], f32r)
            nc.sync.dma_start(out=wout_sb, in_=wout_d.bitcast(f32r))
            d_sb = consts.tile([128, KC, 3], f32)
            nc.sync.dma_start(out=d_sb, in_=d_d)
            cw_sb = consts.tile([128, KC, KCONV], f32)
            nc.sync.dma_start(out=cw_sb, in_=cw_d)
            cb_sb = consts.tile([128, KC], f32)
            nc.sync.dma_start(out=cb_sb, in_=cb_d)
            bout_sb = consts.tile([128, D], f32)
            nc.sync.dma_start(out=bout_sb, in_=bout_d)
            iden_sb = consts.tile([128, 128], f32)
            nc.sync.dma_start(out=iden_sb, in_=iden_d)
            eps_sb = consts.tile([128, 1], f32)
            nc.vector.memset(eps_sb, EPS)
            ones_sb = consts.tile([1, 128], f32)
            nc.vector.memset(ones_sb, 1.0)
            state = consts.tile([128, KC, 3], f32)
            nc.vector.memset(state, 0.0)

            for tb in range(NTB):
                t0 = tb * TB
                # ---- phase A: LN stats + broadcast-normalize xT ----
                xT = xtp.tile([128, DC, TB], f32, name="xT")
                nc.sync.dma_start(out=xT.bitcast(f32r),
                                  in_=xt_d[tb].bitcast(f32r))
                mv4 = lnp.tile([128, 2 * NSUB], f32, name="mv4")
                for s in range(NSUB):
                    xt = xp.tile([128, D], f32, name="xt")
                    nc.sync.dma_start(
                        out=xt, in_=x_d[t0 + s * 128:t0 + (s + 1) * 128, :])
                    stats = lnp.tile([128, 2, 6], f32, name="stats")
                    for g in range(2):
                        nc.vector.bn_stats(
                            out=stats[:, g, :], in_=xt[:, g * 512:(g + 1) * 512])
                    nc.vector.bn_aggr(
                        out=mv4[:, 2 * s:2 * s + 2], in_=stats)
                # transpose [128, 8] -> [8, 128]: rows f = 2s+c (mu, var)
                pt = pst.tile([128, TB], f32, name="bc")
                nc.tensor.transpose(pt[0:8, 0:128], mv4, iden_sb)
                mv8 = rowp.tile([8, 128], f32, name="mv8")
                nc.scalar.copy(out=mv8, in_=pt[0:8, 0:128])
                mrow = rowp.tile([1, 2 * NSUB, 128], f32, name="mrow")
                nc.sync.dma_start(out=mrow, in_=mv8)
                # rank-1 broadcasts: mu_b / var_b [128, TB]
                mu_b = pst.tile([128, TB], f32, name="bc")
                nc.tensor.matmul(mu_b, ones_sb, mrow[:, 0:2 * NSUB:2, :],
                                 start=True, stop=True)
                var_b = pst.tile([128, TB], f32, name="bc")
                nc.tensor.matmul(var_b, ones_sb, mrow[:, 1:2 * NSUB:2, :],
                                 start=True, stop=True)
                rstd_b = rsp.tile([128, TB], f32, name="rstd_b")
                nc.scalar.activation(out=rstd_b, in_=var_b, func=AF.Sqrt,
                                     bias=eps_sb, scale=1.0)
                nc.vector.reciprocal(out=rstd_b, in_=rstd_b)
                for j in range(DC):
                    sc = scp.tile([128, TB], f32, name="sc")
                    nc.vector.tensor_sub(out=sc, in0=xT[:, j, :], in1=mu_b)
                    nc.vector.tensor_mul(out=xT[:, j, :].bitcast(f32r),
                                         in0=sc, in1=rstd_b)

                # ---- phase B: in-proj + gates + conv per channel block ----
                hid2 = hid2p.tile([128, KC, TB], f32r, name="hid2")
                for k in range(KC):
                    wgt = wgp.tile([128, 3 * DC * 128], f32r, name="wgt")
                    nc.sync.dma_start(out=wgt, in_=win_d[k].bitcast(f32r))
                    ps3 = []
                    for t3 in range(3):
                        ps = psm.tile([128, TB], f32, name="mm")
                        for j in range(DC):
                            nc.tensor.matmul(
                                ps,
                                wgt[:, (t3 * DC + j) * 128:
                                    (t3 * DC + j + 1) * 128],
                                xT[:, j, :].bitcast(f32r),
                                start=(j == 0), stop=(j == DC - 1))
                        ps3.append(ps)
                    sigb = sgp.tile([128, TB], f32, name="sigb")
                    nc.scalar.activation(out=sigb, in_=ps3[0], func=AF.Sigmoid,
                                         bias=d_sb[:, k, 0:1], scale=1.0)
                    sigc = sgp.tile([128, TB], f32, name="sigc")
                    nc.scalar.activation(out=sigc, in_=ps3[1], func=AF.Sigmoid,
                                         bias=d_sb[:, k, 1:2], scale=1.0)
                    hidb = hbp.tile([128, TB + 3], f32, name="hidb")
                    nc.vector.tensor_copy(out=hidb[:, 0:3], in_=state[:, k, :])
                    nc.vector.scalar_tensor_tensor(
                        out=hidb[:, 3:TB + 3], in0=ps3[2],
                        scalar=d_sb[:, k, 2:3], in1=sigb,
                        op0=OP.add, op1=OP.mult)
                    nc.vector.tensor_copy(
                        out=state[:, k, :], in_=hidb[:, TB:TB + 3])
                    a0 = accp.tile([128, TB], f32, name="acc0")
                    a1 = accp.tile([128, TB], f32, name="acc1")
                    nc.vector.tensor_scalar_mul(
                        out=a0, in0=hidb[:, 0:TB], scalar1=cw_sb[:, k, 0:1])
                    nc.vector.scalar_tensor_tensor(
                        out=a1, in0=hidb[:, 1:TB + 1],
                        scalar=cw_sb[:, k, 1:2], in1=a0,
                        op0=OP.mult, op1=OP.add)
                    nc.vector.scalar_tensor_tensor(
                        out=a0, in0=hidb[:, 2:TB + 2],
                        scalar=cw_sb[:, k, 2:3], in1=a1,
                        op0=OP.mult, op1=OP.add)
                    nc.vector.scalar_tensor_tensor(
                        out=a1, in0=hidb[:, 3:TB + 3],
                        scalar=cw_sb[:, k, 3:4], in1=a0,
                        op0=OP.mult, op1=OP.add)
                    nc.vector.scalar_tensor_tensor(
                        out=hid2[:, k, :], in0=a1, scalar=cb_sb[:, k:k + 1],
                        in1=sigc, op0=OP.add, op1=OP.mult)

                # ---- phase C: out-proj + bias + residual ----
                for s in range(NSUB):
                    ps0 = psm.tile([128, 512], f32, name="mm")
                    ps1 = psm.tile([128, 512], f32, name="mm")
                    for j in range(KC):
                        nc.tensor.matmul(
                            ps0,
                            hid2[:, j, s * 128:(s + 1) * 128],
                            wout_sb[:, j, 0:512],
                            start=(j == 0), stop=(j == KC - 1))
                        nc.tensor.matmul(
                            ps1,
                            hid2[:, j, s * 128:(s + 1) * 128],
                            wout_sb[:, j, 512:1024],
                            start=(j == 0), stop=(j == KC - 1))
                    for dh, ps in ((0, ps0), (1, ps1)):
                        xr = xrp.tile([128, 512], f32, name="xr")
                        nc.sync.dma_start(
                            out=xr,
                            in_=x_d[t0 + s * 128:t0 + (s + 1) * 128,
                                    dh * 512:(dh + 1) * 512])
                        ot = outp.tile([128, 512], f32, name="ot")
                        nc.vector.tensor_add(
                            out=ot, in0=ps,
                            in1=bout_sb[:, dh * 512:(dh + 1) * 512])
                        nc.vector.tensor_add(out=ot, in0=ot, in1=xr)
                        nc.sync.dma_start(
                            out=out_d[t0 + s * 128:t0 + (s + 1) * 128,
                                      dh * 512:(dh + 1) * 512],
                            in_=ot)
    nc.compile()
    return nc


def _prep_inputs(x, ln_g, ln_b, w_in, b_in, conv_w, conv_b, w_out, b_out):
    x = np.asarray(x, np.float32)
    ln_g = np.asarray(ln_g, np.float32)
    ln_b = np.asarray(ln_b, np.float32)
    w_in = np.asarray(w_in, np.float32)
    b_in = np.asarray(b_in, np.float32)
    conv_w = np.asarray(conv_w, np.float32)
    conv_b = np.asarray(conv_b, np.float32)
    w_out = np.asarray(w_out, np.float32)
    b_out = np.asarray(b_out, np.float32)

    wg = w_in * ln_g[None, :]                       # [3H, D]
    dv = b_in + w_in @ ln_b                         # [3H]
    # [third, k, m, j, p] -> [k, p, third, j, m]
    wblk = wg.reshape(3, KC, 128, DC, 128)
    w_in_proc = np.ascontiguousarray(
        wblk.transpose(1, 4, 0, 3, 2)).reshape(KC, 128, 3 * DC * 128)
    d_proc = np.ascontiguousarray(dv.reshape(3, KC, 128).transpose(2, 1, 0))
    cw_proc = np.ascontiguousarray(
        conv_w.reshape(KC, 128, KCONV).transpose(1, 0, 2))
    cb_proc = np.ascontiguousarray(conv_b.reshape(KC, 128).T)
    w_out_proc = np.ascontiguousarray(
        w_out.T.reshape(KC, 128, D).transpose(1, 0, 2))
    bout_rep = np.ascontiguousarray(np.tile(b_out[None, :], (128, 1)))
    iden = np.eye(128, dtype=np.float32)

    shared = {
        "w_in_proc": w_in_proc, "d_proc": d_proc, "cw_proc": cw_proc,
        "cb_proc": cb_proc, "w_out_proc": w_out_proc, "bout_rep": bout_rep,
        "iden": iden,
    }
    maps = []
    for b in range(B):
        xb = np.ascontiguousarray(x[b])
        # xt_proc[tb, p, j, t] = x[b, 512*tb + t, 128*j + p]
        xt_proc = np.ascontiguousarray(
            xb.T.reshape(DC, 128, NTB, TB).transpose(2, 1, 0, 3))
        maps.append(dict(shared, x=xb, xt_proc=xt_proc))
    return maps


def run(inputs, trace=False):
    _install_ntff_hook()
    import concourse.bass_utils as bu

    bu.upload_artifacts = lambda d: "local://" + d
    if "nc" not in _CACHE:
        _CACHE["nc"] = _build_nc()
    nc = _CACHE["nc"]
    in_maps = _prep_inputs(**inputs)
    res = bu.run_bass_kernel_spmd(nc, in_maps, list(range(N_CORES)),
                                  trace=trace)
    out = np.stack([res.results[b]["out"] for b in range(B)])
    return out.astype(np.float32), res


def kernel(**inputs):
    out, _ = run(inputs, trace=False)
    return out



# revision 16
# speedup vs baseline: 1.3966x; 1.3966x over previous
"""Trainium2 Bass kernel for nn_GatedShortConvBlock.

Full inputs -> full output. Data-parallel over batch: batch b -> core b.
Per-core dataflow (T=4096 tokens, D=1024, H=2048, K=4):
  LN stats on DVE (bn_stats over natural-layout x tiles); mean/var
  transposed via PE and rank-1-broadcast to [128, TB]; normalize+quantize
  on DVE/Pool into fp8 (and bf16) copies of the host-pre-transposed xT.
  in_proj runs as fp8e4 DoubleRow matmuls (2 k-tiles per pass) against
  SBUF-resident fp8 weights; gates on ACT (sigmoid, bias fused);
  depthwise causal conv via scalar_tensor_tensor FMAs alternating
  between DVE and Pool engines; out_proj is bf16 matmuls against
  SBUF-resident bf16 weights producing [d, t]-layout output, epilogue
  fuses +bias +residual in one op; host transposes [D,T] -> [T,D].
All weights live in SBUF for the whole kernel (no weight restreaming);
LN for block tb+1 is software-pipelined under the matmuls of block tb.
"""

import sys
import types

import numpy as np

B, T, D, H = 8, 4096, 1024, 2048
KCONV = 4
N_CORES = 8
TB = 512            # tokens per t-block
NTB = T // TB       # 8
NSUB = TB // 128    # 4
KC = H // 128       # 16 channel blocks
DC = D // 128       # 8 d-chunks
EPS = 1e-5

# Which of the three in_proj thirds (gate_b, gate_c, hidden) use fp8
# DoubleRow; the rest use bf16 streamed weights.
FP8_THIRD = (True, True, True)
# Whether out_proj uses fp8 DoubleRow (hid2 quantized to fp8).
OUT_FP8 = False

_CACHE = {}


def _install_ntff_hook():
    if "antenv.axon_hooks" in sys.modules:
        return
    try:
        import trn_agent_boot.trn_boot as tb

        hook = tb._ntff_profile_via_ctypes("/opt/axon/libaxon_pjrt.so")
        mod = types.ModuleType("antenv.axon_hooks")
        mod.get_axon_ntff_profile_hook = lambda: hook
        mod.set_axon_ntff_profile_hook = lambda h: None
        sys.modules["antenv.axon_hooks"] = mod
    except Exception:
        pass


def _build_nc(ntb=NTB):
    import concourse.bass as bass  # noqa: F401
    import concourse.tile as tile
    from concourse import bacc, mybir

    f32 = mybir.dt.float32
    f32r = mybir.dt.float32r
    bf16 = mybir.dt.bfloat16
    fp8 = mybir.dt.float8e4
    AF = mybir.ActivationFunctionType
    OP = mybir.AluOpType
    DR = mybir.MatmulPerfMode.DoubleRow

    n8 = sum(FP8_THIRD)          # fp8 thirds
    nb = 3 - n8                  # bf16 thirds
    tau8 = [t for t in range(3) if FP8_THIRD[t]]
    taub = [t for t in range(3) if not FP8_THIRD[t]]

    nc = bacc.Bacc("TRN2", target_bir_lowering=False, debug=False,
                   num_devices=N_CORES)
    x_d = nc.dram_tensor("x", [T, D], f32, kind="ExternalInput").ap()
    xt_d = nc.dram_tensor("xt_proc", [NTB, 128, DC, TB], bf16,
                          kind="ExternalInput").ap()
    if n8:
        w8_d = nc.dram_tensor("w8_proc", [128, KC, n8, DC // 2, 2, 128],
                              fp8, kind="ExternalInput").ap()
    if nb:
        wb_d = nc.dram_tensor("wb_proc", [128, KC, nb, DC, 128], bf16,
                              kind="ExternalInput").ap()
    d_d = nc.dram_tensor("d_proc", [128, KC, 3], f32,
                         kind="ExternalInput").ap()
    cw_d = nc.dram_tensor("cw_proc", [128, KC, KCONV], f32,
                          kind="ExternalInput").ap()
    cb_d = nc.dram_tensor("cb_proc", [128, KC], f32,
                          kind="ExternalInput").ap()
    if OUT_FP8:
        wout_d = nc.dram_tensor("wout_proc", [128, KC // 2, 2, DC, 128],
                                fp8, kind="ExternalInput").ap()
    else:
        wout_d = nc.dram_tensor("wout_proc", [128, KC, DC, 128], bf16,
                                kind="ExternalInput").ap()
    boutp_d = nc.dram_tensor("boutp", [128, DC], f32,
                             kind="ExternalInput").ap()
    iden_d = nc.dram_tensor("iden", [128, 128], f32,
                            kind="ExternalInput").ap()
    out_d = nc.dram_tensor("outT", [DC, 128, T], f32,
                           kind="ExternalOutput").ap()

    from contextlib import ExitStack

    with tile.TileContext(nc) as tc, ExitStack() as ctx:
        consts = ctx.enter_context(tc.tile_pool(name="consts", bufs=1))
        xtp = ctx.enter_context(tc.tile_pool(name="xtp", bufs=2))
        x8p = ctx.enter_context(tc.tile_pool(name="x8p", bufs=2))
        if nb:
            xbp = ctx.enter_context(tc.tile_pool(name="xbp", bufs=2))
            wbp = ctx.enter_context(tc.tile_pool(name="wbp", bufs=4))
        xp = ctx.enter_context(tc.tile_pool(name="xp", bufs=2))
        lnp = ctx.enter_context(tc.tile_pool(name="ln", bufs=4))
        rowp = ctx.enter_context(tc.tile_pool(name="rowp", bufs=2))
        rsp = ctx.enter_context(tc.tile_pool(name="rsp", bufs=2))
        scp = ctx.enter_context(tc.tile_pool(name="scp", bufs=2))
        hid2p = ctx.enter_context(tc.tile_pool(name="hid2", bufs=2))
        sgp = ctx.enter_context(tc.tile_pool(name="sg", bufs=4))
        hbp = ctx.enter_context(tc.tile_pool(name="hb", bufs=2))
        accp = ctx.enter_context(tc.tile_pool(name="acc", bufs=2))
        outp = ctx.enter_context(tc.tile_pool(name="outp", bufs=2))
        pb = ctx.enter_context(tc.tile_pool(name="pb", bufs=5, space="PSUM"))
        pc = ctx.enter_context(tc.tile_pool(name="pc", bufs=3, space="PSUM"))

        # ---- resident constants ----
        if n8:
            w8_sb = consts.tile([128, KC, n8, DC // 2, 2, 128], fp8)
            nc.sync.dma_start(out=w8_sb, in_=w8_d)
        if OUT_FP8:
            wout_sb = consts.tile([128, KC // 2, 2, DC, 128], fp8)
        else:
            wout_sb = consts.tile([128, KC, DC, 128], bf16)
        nc.sync.dma_start(out=wout_sb, in_=wout_d)
        d_sb = consts.tile([128, KC, 3], f32)
        nc.sync.dma_start(out=d_sb, in_=d_d)
        cw_sb = consts.tile([128, KC, KCONV], f32)
        nc.sync.dma_start(out=cw_sb, in_=cw_d)
        cb_sb = consts.tile([128, KC], f32)
        nc.sync.dma_start(out=cb_sb, in_=cb_d)
        boutp_sb = consts.tile([128, DC], f32)
        nc.sync.dma_start(out=boutp_sb, in_=boutp_d)
        iden_sb = consts.tile([128, 128], f32)
        nc.sync.dma_start(out=iden_sb, in_=iden_d)
        ones_sb = consts.tile([1, 128], f32)
        nc.vector.memset(ones_sb, 1.0)
        state = consts.tile([128, KC, 3], f32)
        nc.vector.memset(state, 0.0)

        # per-tb rotating tiles, kept across pipeline stages
        cur = {}

        def phaseA_stats(tb):
            """DMA xT + natural x, bn stats -> mv4 [128, 2*NSUB]."""
            t0 = tb * TB
            xT = xtp.tile([128, DC, TB], bf16, name="xT")
            nc.sync.dma_start(out=xT, in_=xt_d[tb])
            mv4 = lnp.tile([128, 2 * NSUB], f32, name="mv4")
            for s in range(NSUB):
                xt = xp.tile([128, D], f32, name="xt")
                nc.sync.dma_start(
                    out=xt, in_=x_d[t0 + s * 128:t0 + (s + 1) * 128, :])
                stats = lnp.tile([128, 2, 6], f32, name="stats")
                eng = nc.vector
                for g in range(2):
                    eng.bn_stats(
                        out=stats[:, g, :], in_=xt[:, g * 512:(g + 1) * 512])
                # deinterleaved layout: col [s] = mean, [NSUB+s] = var
                eng.bn_aggr(out=mv4[:, s::NSUB], in_=stats)
            cur[("xT", tb)] = xT
            cur[("mv4", tb)] = mv4

        def phaseA_transpose(tb):
            mv4 = cur.pop(("mv4", tb))
            pt = pb.tile([128, TB], f32, name="pbt")
            nc.tensor.transpose(pt[0:8, 0:128], mv4, iden_sb)
            mv8 = rowp.tile([8, 128], f32, name="mv8")
            nc.scalar.copy(out=mv8, in_=pt[0:8, 0:128])
            # Newton-iterate y -> 1/sqrt(var+eps) on rows 4..7 (DVE only:
            # no Sqrt on ACT => act-func table never switches off sigmoid).
            vp = lnp.tile([8, 128], f32, name="vp")
            yt = lnp.tile([8, 128], f32, name="yt")
            at = lnp.tile([8, 128], f32, name="at")
            v, y, a = vp, yt, at
            nc.vector.tensor_scalar_add(out=v, in0=mv8, scalar1=EPS)
            nc.vector.tensor_scalar(out=y, in0=v, scalar1=-0.5,
                                    scalar2=1.5, op0=OP.mult, op1=OP.add)
            for _ in range(3):
                nc.vector.tensor_mul(out=a, in0=y, in1=y)
                nc.vector.tensor_mul(out=a, in0=a, in1=v)
                nc.vector.tensor_scalar(out=a, in0=a, scalar1=-0.5,
                                        scalar2=1.5, op0=OP.mult, op1=OP.add)
                nc.vector.tensor_mul(out=y, in0=y, in1=a)
            mrow = rowp.tile([1, NSUB, 128], f32, name="mrow")
            nc.sync.dma_start(out=mrow, in_=mv8[0:NSUB, :])
            rrow = rowp.tile([1, NSUB, 128], f32, name="rrow")
            nc.sync.dma_start(out=rrow, in_=yt[NSUB:2 * NSUB, :])
            cur[("mrow", tb)] = mrow
            cur[("rrow", tb)] = rrow

        def phaseA_broadcast(tb):
            mrow = cur.pop(("mrow", tb))
            rrow = cur.pop(("rrow", tb))
            mu_b = pc.tile([128, TB], f32, name="pct")
            nc.tensor.matmul(mu_b, ones_sb.bitcast(f32r),
                             mrow.bitcast(f32r), start=True, stop=True)
            rs_b = pc.tile([128, TB], f32, name="pct")
            nc.tensor.matmul(rs_b, ones_sb.bitcast(f32r),
                             rrow.bitcast(f32r), start=True, stop=True)
            rstd_b = rsp.tile([128, TB], f32, name="rstd_b")
            nc.scalar.activation(out=rstd_b, in_=rs_b, func=AF.Identity,
                                 bias=0.0, scale=1.0)
            mu_s = rsp.tile([128, TB], f32, name="mu_s")
            nc.scalar.activation(out=mu_s, in_=mu_b, func=AF.Identity,
                                 bias=0.0, scale=1.0)
            cur[("mu_s", tb)] = mu_s
            cur[("rstd_b", tb)] = rstd_b

        def phaseA_normalize(tb):
            mu_s = cur.pop(("mu_s", tb))
            rstd_b = cur.pop(("rstd_b", tb))
            xT = cur[("xT", tb)]
            xn8 = None
            xnb = None
            if n8:
                xn8 = x8p.tile([128, DC, TB], fp8, name="xn8")
            if nb:
                xnb = xbp.tile([128, DC, TB], bf16, name="xnb")
            for j in range(DC):
                eng = nc.gpsimd
                sc = scp.tile([128, TB], f32, name="sc")
                eng.tensor_sub(out=sc, in0=xT[:, j, :], in1=mu_s)
                if n8:
                    eng.tensor_mul(out=xn8[:, j, :], in0=sc, in1=rstd_b)
                if nb:
                    eng.tensor_mul(out=xnb[:, j, :], in0=sc, in1=rstd_b)
            cur[("xn8", tb)] = xn8
            cur[("xnb", tb)] = xnb

        def phaseB_k(tb, k, hid2):
            xn8 = cur.get(("xn8", tb))
            xnb = cur.get(("xnb", tb))
            ps = [None] * 3
            for t8, tau in enumerate(tau8):
                p = pb.tile([128, TB], f32, name="pbt")
                for jj in range(DC // 2):
                    nc.tensor.matmul(
                        p, w8_sb[:, k, t8, jj],
                        xn8[:, 2 * jj:2 * jj + 2, :],
                        start=(jj == 0), stop=(jj == DC // 2 - 1),
                        perf_mode=DR)
                ps[tau] = p
            if nb:
                wgt = cur.pop(("wb", tb, k))
                for tb_i, tau in enumerate(taub):
                    p = pb.tile([128, TB], f32, name="pbt")
                    for j in range(DC):
                        nc.tensor.matmul(
                            p, wgt[:, tb_i, j], xnb[:, j, :],
                            start=(j == 0), stop=(j == DC - 1))
                    ps[tau] = p
            sigb = sgp.tile([128, TB], f32, name="sigb")
            nc.scalar.activation(out=sigb, in_=ps[0], func=AF.Sigmoid,
                                 bias=d_sb[:, k, 0:1], scale=1.0)
            sigc = sgp.tile([128, TB], f32, name="sigc")
            nc.scalar.activation(out=sigc, in_=ps[1], func=AF.Sigmoid,
                                 bias=d_sb[:, k, 1:2], scale=1.0)
            eng = nc.vector
            hidb = hbp.tile([128, TB + 3], f32, name="hidb")
            eng.tensor_copy(out=hidb[:, 0:3], in_=state[:, k, :])
            eng.scalar_tensor_tensor(
                out=hidb[:, 3:TB + 3], in0=ps[2],
                scalar=d_sb[:, k, 2:3], in1=sigb,
                op0=OP.add, op1=OP.mult)
            eng.tensor_copy(out=state[:, k, :], in_=hidb[:, TB:TB + 3])
            a0 = accp.tile([128, TB], f32, name="acc0")
            a1 = accp.tile([128, TB], f32, name="acc1")
            eng.tensor_scalar_mul(
                out=a0, in0=hidb[:, 0:TB], scalar1=cw_sb[:, k, 0:1])
            eng.scalar_tensor_tensor(
                out=a1, in0=hidb[:, 1:TB + 1],
                scalar=cw_sb[:, k, 1:2], in1=a0, op0=OP.mult, op1=OP.add)
            eng.scalar_tensor_tensor(
                out=a0, in0=hidb[:, 2:TB + 2],
                scalar=cw_sb[:, k, 2:3], in1=a1, op0=OP.mult, op1=OP.add)
            eng.scalar_tensor_tensor(
                out=a1, in0=hidb[:, 3:TB + 3],
                scalar=cw_sb[:, k, 3:4], in1=a0, op0=OP.mult, op1=OP.add)
            eng.scalar_tensor_tensor(
                out=hid2[:, k, :], in0=a1, scalar=cb_sb[:, k:k + 1],
                in1=sigc, op0=OP.add, op1=OP.mult)

        def stream_wb(tb, k):
            if not nb:
                return
            wgt = wbp.tile([128, nb, DC, 128], bf16, name="wgt")
            nc.sync.dma_start(out=wgt, in_=wb_d[:, k])
            cur[("wb", tb, k)] = wgt

        def phaseC(tb, hid2):
            t0 = tb * TB
            xT = cur.pop(("xT", tb))
            for j in range(DC):
                p = pc.tile([128, TB], f32, name="pct")
                if OUT_FP8:
                    for kk in range(KC // 2):
                        nc.tensor.matmul(
                            p, wout_sb[:, kk, :, j],
                            hid2[:, 2 * kk:2 * kk + 2, :],
                            start=(kk == 0), stop=(kk == KC // 2 - 1),
                            perf_mode=DR)
                else:
                    for k in range(KC):
                        nc.tensor.matmul(
                            p, wout_sb[:, k, j], hid2[:, k, :],
                            start=(k == 0), stop=(k == KC - 1))
                ot = outp.tile([128, TB], f32, name="ot")
                nc.scalar.activation(out=ot, in_=p, func=AF.Identity,
                                     bias=boutp_sb[:, j:j + 1], scale=1.0)
                nc.gpsimd.tensor_add(out=ot, in0=ot, in1=xT[:, j, :])
                nc.sync.dma_start(out=out_d[j, :, t0:t0 + TB], in_=ot)

        # ---- software-pipelined main loop ----
        phaseA_stats(0)
        phaseA_transpose(0)
        phaseA_broadcast(0)
        phaseA_normalize(0)
        h2dt = fp8 if OUT_FP8 else bf16
        for tb in range(ntb):
            hid2 = hid2p.tile([128, KC, TB], h2dt, name="hid2")
            nxt = tb + 1
            for k in range(KC):
                if nb and k == 0:
                    stream_wb(tb, 0)
                if nb and k < KC - 1:
                    stream_wb(tb, k + 1)
                phaseB_k(tb, k, hid2)
                if nxt < ntb:
                    if k == 3:
                        phaseA_stats(nxt)
                    elif k == 6:
                        phaseA_transpose(nxt)
                    elif k == 7:
                        phaseA_broadcast(nxt)
                    elif k == KC - 1:
                        phaseA_normalize(nxt)
            cur.pop(("xn8", tb), None)
            cur.pop(("xnb", tb), None)
            phaseC(tb, hid2)
    nc.compile()
    return nc


def _prep_inputs(x, ln_g, ln_b, w_in, b_in, conv_w, conv_b, w_out, b_out,
                 ntb=NTB):
    import ml_dtypes

    f8 = ml_dtypes.float8_e4m3
    x = np.asarray(x, np.float32)
    ln_g = np.asarray(ln_g, np.float32)
    ln_b = np.asarray(ln_b, np.float32)
    w_in = np.asarray(w_in, np.float32)
    b_in = np.asarray(b_in, np.float32)
    conv_w = np.asarray(conv_w, np.float32)
    conv_b = np.asarray(conv_b, np.float32)
    w_out = np.asarray(w_out, np.float32)
    b_out = np.asarray(b_out, np.float32)

    tau8 = [t for t in range(3) if FP8_THIRD[t]]
    taub = [t for t in range(3) if not FP8_THIRD[t]]

    wg = w_in * ln_g[None, :]                       # [3H, D]
    dv = b_in + w_in @ ln_b                         # [3H]
    # [tau, k, m, jj, i, p] -> [p, k, tau, jj, i, m]
    wblk = wg.reshape(3, KC, 128, DC // 2, 2, 128)
    wperm = wblk.transpose(5, 1, 0, 3, 4, 2)        # [p, k, tau, jj, i, m]
    shared = {}
    if tau8:
        shared["w8_proc"] = np.ascontiguousarray(
            wperm[:, :, tau8]).astype(f8)
    if taub:
        wb = wg.reshape(3, KC, 128, DC, 128).transpose(4, 1, 0, 3, 2)
        shared["wb_proc"] = np.ascontiguousarray(
            wb[:, :, taub]).astype(ml_dtypes.bfloat16)
    shared["d_proc"] = np.ascontiguousarray(
        dv.reshape(3, KC, 128).transpose(2, 1, 0))
    shared["cw_proc"] = np.ascontiguousarray(
        conv_w.reshape(KC, 128, KCONV).transpose(1, 0, 2))
    shared["cb_proc"] = np.ascontiguousarray(conv_b.reshape(KC, 128).T)
    # wout_proc[p, k, j, m] = w_out[128j+m, 128k+p]
    wo = w_out.reshape(DC, 128, KC, 128).transpose(3, 2, 0, 1)
    if OUT_FP8:
        shared["wout_proc"] = np.ascontiguousarray(
            wo.reshape(128, KC // 2, 2, DC, 128)).astype(f8)
    else:
        shared["wout_proc"] = np.ascontiguousarray(
            wo).astype(ml_dtypes.bfloat16)
    shared["boutp"] = np.ascontiguousarray(b_out.reshape(DC, 128).T)
    shared["iden"] = np.eye(128, dtype=np.float32)

    maps = []
    for b in range(B):
        xb = np.ascontiguousarray(x[b])
        # xt_proc[tb, p, j, t] = x[b, 512*tb + t, 128*j + p]
        xt_proc = np.ascontiguousarray(
            xb.T.reshape(DC, 128, NTB, TB).transpose(2, 1, 0, 3)).astype(
                ml_dtypes.bfloat16)
        maps.append(dict(shared, x=xb, xt_proc=xt_proc))
    return maps


def run(inputs, trace=False):
    _install_ntff_hook()
    import concourse.bass_utils as bu

    bu.upload_artifacts = lambda d: "local://" + d
    if "nc" not in _CACHE:
        _CACHE["nc"] = _build_nc()
    nc = _CACHE["nc"]
    in_maps = _prep_inputs(**inputs)
    res = bu.run_bass_kernel_spmd(nc, in_maps, list(range(N_CORES)),
                                  trace=trace)
    out = np.stack([
        np.ascontiguousarray(
            res.results[b]["outT"].reshape(D, T).T)
        for b in range(B)
    ])
    return out.astype(np.float32), res


def kernel(**inputs):
    out, _ = run(inputs, trace=False)
    return out


# revision 20
# speedup vs baseline: 1.5738x; 1.1269x over previous
"""Trainium2 Bass kernel for nn_GatedShortConvBlock.

Full inputs -> full output. Data-parallel over batch: batch b -> core b.
Per-core dataflow (T=4096 tokens, D=1024, H=2048, K=4):
  LN stats on DVE (bn_stats over natural-layout x tiles); mean/var
  transposed via PE and rank-1-broadcast to [128, TB]; normalize+quantize
  on DVE/Pool into fp8 (and bf16) copies of the host-pre-transposed xT.
  in_proj runs as fp8e4 DoubleRow matmuls (2 k-tiles per pass) against
  SBUF-resident fp8 weights; gates on ACT (sigmoid, bias fused);
  depthwise causal conv via scalar_tensor_tensor FMAs alternating
  between DVE and Pool engines; out_proj is bf16 matmuls against
  SBUF-resident bf16 weights producing [d, t]-layout output, epilogue
  fuses +bias +residual in one op; host transposes [D,T] -> [T,D].
All weights live in SBUF for the whole kernel (no weight restreaming);
LN for block tb+1 is software-pipelined under the matmuls of block tb.
"""

import sys
import types

import numpy as np

B, T, D, H = 8, 4096, 1024, 2048
KCONV = 4
N_CORES = 8
TB = 512            # tokens per t-block
NTB = T // TB       # 8
NSUB = TB // 128    # 4
KC = H // 128       # 16 channel blocks
DC = D // 128       # 8 d-chunks
EPS = 1e-5

# Which of the three in_proj thirds (gate_b, gate_c, hidden) use fp8
# DoubleRow; the rest use bf16 streamed weights.
FP8_THIRD = (True, True, True)
# Whether out_proj uses fp8 DoubleRow (hid2 quantized to fp8).
OUT_FP8 = False

_CACHE = {}


def _install_ntff_hook():
    if "antenv.axon_hooks" in sys.modules:
        return
    try:
        import trn_agent_boot.trn_boot as tb

        hook = tb._ntff_profile_via_ctypes("/opt/axon/libaxon_pjrt.so")
        mod = types.ModuleType("antenv.axon_hooks")
        mod.get_axon_ntff_profile_hook = lambda: hook
        mod.set_axon_ntff_profile_hook = lambda h: None
        sys.modules["antenv.axon_hooks"] = mod
    except Exception:
        pass


def _build_nc(ntb=NTB):
    import concourse.bass as bass  # noqa: F401
    import concourse.tile as tile
    from concourse import bacc, mybir

    f32 = mybir.dt.float32
    f32r = mybir.dt.float32r
    bf16 = mybir.dt.bfloat16
    fp8 = mybir.dt.float8e4
    AF = mybir.ActivationFunctionType
    OP = mybir.AluOpType
    DR = mybir.MatmulPerfMode.DoubleRow

    n8 = sum(FP8_THIRD)          # fp8 thirds
    nb = 3 - n8                  # bf16 thirds
    tau8 = [t for t in range(3) if FP8_THIRD[t]]
    taub = [t for t in range(3) if not FP8_THIRD[t]]

    nc = bacc.Bacc("TRN2", target_bir_lowering=False, debug=False,
                   num_devices=N_CORES)
    x_d = nc.dram_tensor("x", [T, D], f32, kind="ExternalInput").ap()
    xt_d = nc.dram_tensor("xt_proc", [NTB, 128, DC, TB], bf16,
                          kind="ExternalInput").ap()
    if n8:
        w8_d = nc.dram_tensor("w8_proc", [128, KC, n8, DC // 2, 2, 128],
                              fp8, kind="ExternalInput").ap()
    if nb:
        wb_d = nc.dram_tensor("wb_proc", [128, KC, nb, DC, 128], bf16,
                              kind="ExternalInput").ap()
    d_d = nc.dram_tensor("d_proc", [128, KC, 3], f32,
                         kind="ExternalInput").ap()
    cwd_d = nc.dram_tensor("cwd_proc", [128, KC, KCONV, 128], bf16,
                           kind="ExternalInput").ap()
    cb_d = nc.dram_tensor("cb_proc", [128, KC], f32,
                          kind="ExternalInput").ap()
    if OUT_FP8:
        wout_d = nc.dram_tensor("wout_proc", [128, KC // 2, 2, DC, 128],
                                fp8, kind="ExternalInput").ap()
    else:
        wout_d = nc.dram_tensor("wout_proc", [128, KC, DC, 128], bf16,
                                kind="ExternalInput").ap()
    boutp_d = nc.dram_tensor("boutp", [128, DC], f32,
                             kind="ExternalInput").ap()
    iden_d = nc.dram_tensor("iden", [128, 128], f32,
                            kind="ExternalInput").ap()
    out_d = nc.dram_tensor("outT", [DC, 128, T], f32,
                           kind="ExternalOutput").ap()

    from contextlib import ExitStack

    with tile.TileContext(nc) as tc, ExitStack() as ctx:
        consts = ctx.enter_context(tc.tile_pool(name="consts", bufs=1))
        xtp = ctx.enter_context(tc.tile_pool(name="xtp", bufs=2))
        x8p = ctx.enter_context(tc.tile_pool(name="x8p", bufs=2))
        if nb:
            xbp = ctx.enter_context(tc.tile_pool(name="xbp", bufs=2))
            wbp = ctx.enter_context(tc.tile_pool(name="wbp", bufs=4))
        xp = ctx.enter_context(tc.tile_pool(name="xp", bufs=2))
        lnp = ctx.enter_context(tc.tile_pool(name="ln", bufs=4))
        rowp = ctx.enter_context(tc.tile_pool(name="rowp", bufs=2))
        rsp = ctx.enter_context(tc.tile_pool(name="rsp", bufs=2))
        scp = ctx.enter_context(tc.tile_pool(name="scp", bufs=2))
        hid2p = ctx.enter_context(tc.tile_pool(name="hid2", bufs=2))
        sgp = ctx.enter_context(tc.tile_pool(name="sg", bufs=4))
        hbp = ctx.enter_context(tc.tile_pool(name="hb", bufs=4))
        outp = ctx.enter_context(tc.tile_pool(name="outp", bufs=2))
        pb = ctx.enter_context(tc.tile_pool(name="pb", bufs=5, space="PSUM"))
        pc = ctx.enter_context(tc.tile_pool(name="pc", bufs=3, space="PSUM"))

        # ---- resident constants ----
        if n8:
            w8_sb = consts.tile([128, KC, n8, DC // 2, 2, 128], fp8)
            nc.sync.dma_start(out=w8_sb, in_=w8_d)
        if OUT_FP8:
            wout_sb = consts.tile([128, KC // 2, 2, DC, 128], fp8)
        else:
            wout_sb = consts.tile([128, KC, DC, 128], bf16)
        nc.sync.dma_start(out=wout_sb, in_=wout_d)
        d_sb = consts.tile([128, KC, 3], f32)
        nc.sync.dma_start(out=d_sb, in_=d_d)
        cwd_sb = consts.tile([128, KC, KCONV, 128], bf16)
        nc.sync.dma_start(out=cwd_sb, in_=cwd_d)
        cb_sb = consts.tile([128, KC], f32)
        nc.sync.dma_start(out=cb_sb, in_=cb_d)
        boutp_sb = consts.tile([128, DC], f32)
        nc.sync.dma_start(out=boutp_sb, in_=boutp_d)
        iden_sb = consts.tile([128, 128], f32)
        nc.sync.dma_start(out=iden_sb, in_=iden_d)
        ones_sb = consts.tile([1, 128], f32)
        nc.vector.memset(ones_sb, 1.0)
        state = consts.tile([128, KC, 3], bf16)
        nc.vector.memset(state, 0.0)

        # per-tb rotating tiles, kept across pipeline stages
        cur = {}

        def phaseA_stats(tb):
            """DMA xT + natural x, bn stats -> mv4 [128, 2*NSUB]."""
            t0 = tb * TB
            xT = xtp.tile([128, DC, TB], bf16, name="xT")
            nc.sync.dma_start(out=xT, in_=xt_d[tb])
            mv4 = lnp.tile([128, 2 * NSUB], f32, name="mv4")
            for s in range(NSUB):
                xt = xp.tile([128, D], f32, name="xt")
                nc.sync.dma_start(
                    out=xt, in_=x_d[t0 + s * 128:t0 + (s + 1) * 128, :])
                stats = lnp.tile([128, 2, 6], f32, name="stats")
                eng = nc.vector
                for g in range(2):
                    eng.bn_stats(
                        out=stats[:, g, :], in_=xt[:, g * 512:(g + 1) * 512])
                # deinterleaved layout: col [s] = mean, [NSUB+s] = var
                eng.bn_aggr(out=mv4[:, s::NSUB], in_=stats)
            cur[("xT", tb)] = xT
            cur[("mv4", tb)] = mv4

        def phaseA_transpose(tb):
            mv4 = cur.pop(("mv4", tb))
            pt = pb.tile([128, TB], f32, name="pbt")
            nc.tensor.transpose(pt[0:8, 0:128], mv4, iden_sb)
            mv8 = rowp.tile([8, 128], f32, name="mv8")
            nc.scalar.copy(out=mv8, in_=pt[0:8, 0:128])
            # Newton-iterate y -> 1/sqrt(var+eps) on rows 4..7 (DVE only:
            # no Sqrt on ACT => act-func table never switches off sigmoid).
            vp = lnp.tile([8, 128], f32, name="vp")
            yt = lnp.tile([8, 128], f32, name="yt")
            at = lnp.tile([8, 128], f32, name="at")
            v, y, a = vp, yt, at
            nc.vector.tensor_scalar_add(out=v, in0=mv8, scalar1=EPS)
            nc.vector.tensor_scalar(out=y, in0=v, scalar1=-0.5,
                                    scalar2=1.5, op0=OP.mult, op1=OP.add)
            for _ in range(3):
                nc.vector.tensor_mul(out=a, in0=y, in1=y)
                nc.vector.tensor_mul(out=a, in0=a, in1=v)
                nc.vector.tensor_scalar(out=a, in0=a, scalar1=-0.5,
                                        scalar2=1.5, op0=OP.mult, op1=OP.add)
                nc.vector.tensor_mul(out=y, in0=y, in1=a)
            mrow = rowp.tile([1, NSUB, 128], f32, name="mrow")
            nc.sync.dma_start(out=mrow, in_=mv8[0:NSUB, :])
            rrow = rowp.tile([1, NSUB, 128], f32, name="rrow")
            nc.sync.dma_start(out=rrow, in_=yt[NSUB:2 * NSUB, :])
            cur[("mrow", tb)] = mrow
            cur[("rrow", tb)] = rrow

        def phaseA_broadcast(tb):
            mrow = cur.pop(("mrow", tb))
            rrow = cur.pop(("rrow", tb))
            mu_b = pc.tile([128, TB], f32, name="pct")
            nc.tensor.matmul(mu_b, ones_sb.bitcast(f32r),
                             mrow.bitcast(f32r), start=True, stop=True)
            rs_b = pc.tile([128, TB], f32, name="pct")
            nc.tensor.matmul(rs_b, ones_sb.bitcast(f32r),
                             rrow.bitcast(f32r), start=True, stop=True)
            rstd_b = rsp.tile([128, TB], f32, name="rstd_b")
            nc.scalar.activation(out=rstd_b, in_=rs_b, func=AF.Identity,
                                 bias=0.0, scale=1.0)
            mu_s = rsp.tile([128, TB], f32, name="mu_s")
            nc.scalar.activation(out=mu_s, in_=mu_b, func=AF.Identity,
                                 bias=0.0, scale=1.0)
            cur[("mu_s", tb)] = mu_s
            cur[("rstd_b", tb)] = rstd_b

        def phaseA_normalize(tb):
            mu_s = cur.pop(("mu_s", tb))
            rstd_b = cur.pop(("rstd_b", tb))
            xT = cur[("xT", tb)]
            xn8 = None
            xnb = None
            if n8:
                xn8 = x8p.tile([128, DC, TB], fp8, name="xn8")
            if nb:
                xnb = xbp.tile([128, DC, TB], bf16, name="xnb")
            for j in range(DC):
                eng = nc.gpsimd
                sc = scp.tile([128, TB], bf16, name="sc")
                eng.tensor_sub(out=sc, in0=xT[:, j, :], in1=mu_s)
                if n8:
                    eng.tensor_mul(out=xn8[:, j, :], in0=sc, in1=rstd_b)
                if nb:
                    eng.tensor_mul(out=xnb[:, j, :], in0=sc, in1=rstd_b)
            cur[("xn8", tb)] = xn8
            cur[("xnb", tb)] = xnb

        def phaseB_k(tb, k, hid2):
            xn8 = cur.get(("xn8", tb))
            xnb = cur.get(("xnb", tb))
            ps = [None] * 3
            for t8, tau in enumerate(tau8):
                p = pb.tile([128, TB], f32, name="pbt")
                for jj in range(DC // 2):
                    nc.tensor.matmul(
                        p, w8_sb[:, k, t8, jj],
                        xn8[:, 2 * jj:2 * jj + 2, :],
                        start=(jj == 0), stop=(jj == DC // 2 - 1),
                        perf_mode=DR)
                ps[tau] = p
            if nb:
                wgt = cur.pop(("wb", tb, k))
                for tb_i, tau in enumerate(taub):
                    p = pb.tile([128, TB], f32, name="pbt")
                    for j in range(DC):
                        nc.tensor.matmul(
                            p, wgt[:, tb_i, j], xnb[:, j, :],
                            start=(j == 0), stop=(j == DC - 1))
                    ps[tau] = p
            sigb = sgp.tile([128, TB], bf16, name="sigb")
            nc.scalar.activation(out=sigb, in_=ps[0], func=AF.Sigmoid,
                                 bias=d_sb[:, k, 0:1], scale=1.0)
            sigc = sgp.tile([128, TB], bf16, name="sigc")
            nc.scalar.activation(out=sigc, in_=ps[1], func=AF.Sigmoid,
                                 bias=d_sb[:, k, 1:2], scale=1.0)
            hidb = hbp.tile([128, TB + 3], bf16, name="hidb")
            nc.gpsimd.tensor_copy(out=hidb[:, 0:3], in_=state[:, k, :])
            nc.vector.scalar_tensor_tensor(
                out=hidb[:, 3:TB + 3], in0=ps[2],
                scalar=d_sb[:, k, 2:3], in1=sigb,
                op0=OP.add, op1=OP.mult)
            nc.gpsimd.tensor_copy(out=state[:, k, :],
                                  in_=hidb[:, TB:TB + 3])
            cur[("hidb", tb, k)] = hidb
            cur[("sigc", tb, k)] = sigc

        def conv_k(tb, k, hid2):
            """Depthwise conv as 4 PSUM-accumulated diag matmuls on PE."""
            hidb = cur.pop(("hidb", tb, k))
            sigc = cur.pop(("sigc", tb, k))
            psC = pb.tile([128, TB], f32, name="pbt")
            for kp in range(KCONV):
                nc.tensor.matmul(psC, cwd_sb[:, k, kp], hidb[:, kp:kp + TB],
                                 start=(kp == 0), stop=(kp == KCONV - 1))
            nc.vector.scalar_tensor_tensor(
                out=hid2[:, k, :], in0=psC, scalar=cb_sb[:, k:k + 1],
                in1=sigc, op0=OP.add, op1=OP.mult)

        def stream_wb(tb, k):
            if not nb:
                return
            wgt = wbp.tile([128, nb, DC, 128], bf16, name="wgt")
            nc.sync.dma_start(out=wgt, in_=wb_d[:, k])
            cur[("wb", tb, k)] = wgt

        def phaseC(tb, hid2):
            t0 = tb * TB
            xT = cur.pop(("xT", tb))
            for j in range(DC):
                p = pc.tile([128, TB], f32, name="pct")
                if OUT_FP8:
                    for kk in range(KC // 2):
                        nc.tensor.matmul(
                            p, wout_sb[:, kk, :, j],
                            hid2[:, 2 * kk:2 * kk + 2, :],
                            start=(kk == 0), stop=(kk == KC // 2 - 1),
                            perf_mode=DR)
                else:
                    for k in range(KC):
                        nc.tensor.matmul(
                            p, wout_sb[:, k, j], hid2[:, k, :],
                            start=(k == 0), stop=(k == KC - 1))
                ot = outp.tile([128, TB], f32, name="ot")
                nc.scalar.activation(out=ot, in_=p, func=AF.Identity,
                                     bias=boutp_sb[:, j:j + 1], scale=1.0)
                nc.gpsimd.tensor_add(out=ot, in0=ot, in1=xT[:, j, :])
                nc.sync.dma_start(out=out_d[j, :, t0:t0 + TB], in_=ot)

        # ---- software-pipelined main loop ----
        phaseA_stats(0)
        phaseA_transpose(0)
        phaseA_broadcast(0)
        phaseA_normalize(0)
        h2dt = fp8 if OUT_FP8 else bf16
        for tb in range(ntb):
            hid2 = hid2p.tile([128, KC, TB], h2dt, name="hid2")
            nxt = tb + 1
            LAG = 2
            for k in range(KC):
                if nb and k == 0:
                    stream_wb(tb, 0)
                if nb and k < KC - 1:
                    stream_wb(tb, k + 1)
                phaseB_k(tb, k, hid2)
                if k >= LAG:
                    conv_k(tb, k - LAG, hid2)
                if nxt < ntb:
                    if k == 3:
                        phaseA_stats(nxt)
                    elif k == 6:
                        phaseA_transpose(nxt)
                    elif k == 7:
                        phaseA_broadcast(nxt)
                    elif k == KC - 1:
                        phaseA_normalize(nxt)
            for k in range(KC - LAG, KC):
                conv_k(tb, k, hid2)
            cur.pop(("xn8", tb), None)
            cur.pop(("xnb", tb), None)
            phaseC(tb, hid2)
    nc.compile()
    return nc


def _prep_inputs(x, ln_g, ln_b, w_in, b_in, conv_w, conv_b, w_out, b_out,
                 ntb=NTB):
    import ml_dtypes

    f8 = ml_dtypes.float8_e4m3
    x = np.asarray(x, np.float32)
    ln_g = np.asarray(ln_g, np.float32)
    ln_b = np.asarray(ln_b, np.float32)
    w_in = np.asarray(w_in, np.float32)
    b_in = np.asarray(b_in, np.float32)
    conv_w = np.asarray(conv_w, np.float32)
    conv_b = np.asarray(conv_b, np.float32)
    w_out = np.asarray(w_out, np.float32)
    b_out = np.asarray(b_out, np.float32)

    tau8 = [t for t in range(3) if FP8_THIRD[t]]
    taub = [t for t in range(3) if not FP8_THIRD[t]]

    wg = w_in * ln_g[None, :]                       # [3H, D]
    dv = b_in + w_in @ ln_b                         # [3H]
    # [tau, k, m, jj, i, p] -> [p, k, tau, jj, i, m]
    wblk = wg.reshape(3, KC, 128, DC // 2, 2, 128)
    wperm = wblk.transpose(5, 1, 0, 3, 4, 2)        # [p, k, tau, jj, i, m]
    shared = {}
    if tau8:
        shared["w8_proc"] = np.ascontiguousarray(
            wperm[:, :, tau8]).astype(f8)
    if taub:
        wb = wg.reshape(3, KC, 128, DC, 128).transpose(4, 1, 0, 3, 2)
        shared["wb_proc"] = np.ascontiguousarray(
            wb[:, :, taub]).astype(ml_dtypes.bfloat16)
    shared["d_proc"] = np.ascontiguousarray(
        dv.reshape(3, KC, 128).transpose(2, 1, 0))
    # cwd_proc[p, k, kp, m] = conv_w[128k+m, 0, kp] if m == p else 0
    cw = conv_w.reshape(KC, 128, KCONV)          # [k, m, kp]
    cwd = np.zeros((128, KC, KCONV, 128), np.float32)
    pi = np.arange(128)
    cwd[pi, :, :, pi] = cw.transpose(1, 0, 2)    # [m=p, k, kp]
    shared["cwd_proc"] = cwd.astype(ml_dtypes.bfloat16)
    shared["cb_proc"] = np.ascontiguousarray(conv_b.reshape(KC, 128).T)
    # wout_proc[p, k, j, m] = w_out[128j+m, 128k+p]
    wo = w_out.reshape(DC, 128, KC, 128).transpose(3, 2, 0, 1)
    if OUT_FP8:
        shared["wout_proc"] = np.ascontiguousarray(
            wo.reshape(128, KC // 2, 2, DC, 128)).astype(f8)
    else:
        shared["wout_proc"] = np.ascontiguousarray(
            wo).astype(ml_dtypes.bfloat16)
    shared["boutp"] = np.ascontiguousarray(b_out.reshape(DC, 128).T)
    shared["iden"] = np.eye(128, dtype=np.float32)

    maps = []
    for b in range(B):
        xb = np.ascontiguousarray(x[b])
        # xt_proc[tb, p, j, t] = x[b, 512*tb + t, 128*j + p]
        xt_proc = np.ascontiguousarray(
            xb.T.reshape(DC, 128, NTB, TB).transpose(2, 1, 0, 3)).astype(
                ml_dtypes.bfloat16)
        maps.append(dict(shared, x=xb, xt_proc=xt_proc))
    return maps


def run(inputs, trace=False):
    _install_ntff_hook()
    import concourse.bass_utils as bu

    bu.upload_artifacts = lambda d: "local://" + d
    if "nc" not in _CACHE:
        _CACHE["nc"] = _build_nc()
    nc = _CACHE["nc"]
    in_maps = _prep_inputs(**inputs)
    res = bu.run_bass_kernel_spmd(nc, in_maps, list(range(N_CORES)),
                                  trace=trace)
    out = np.stack([
        np.ascontiguousarray(
            res.results[b]["outT"].reshape(D, T).T)
        for b in range(B)
    ])
    return out.astype(np.float32), res


def kernel(**inputs):
    out, _ = run(inputs, trace=False)
    return out


# revision 47
# speedup vs baseline: 1.7660x; 1.1221x over previous
"""Trainium2 Bass kernel for nn_GatedShortConvBlock.

Full inputs -> full output. Data-parallel over batch: batch b -> core b.
Per-core dataflow (T=4096 tokens, D=1024, H=2048, K=4), per 512-token
block:
  LN stats via bn_stats on DVE over natural-layout x; 1/sqrt(var+eps)
  via a 4-step Newton iteration on DVE (keeps ACT pinned to the sigmoid
  act-function table all kernel); mean/rstd rank-1-broadcast by PE and
  evacuated to SBUF by ACT; normalize+fp8-quantize on Pool.
  in_proj: fp8e4 DoubleRow matmuls (256-deep contraction per pass,
  2x bf16 rate) against SBUF-resident fp8 weights (LayerNorm gamma/beta
  folded into weights/bias on the host). Gates: sigmoid on ACT with
  fused per-channel bias, bf16 out. Gated hidden via one stt on DVE.
  Depthwise causal conv: 4 PSUM-accumulated PE matmuls per channel
  block with diagonal bf16 stationaries against free-dim-shifted
  windows of the gated signal; (+conv bias) * sigmoid gate fused in one
  DVE stt. out_proj: bf16 matmuls into [d, t]-layout output; epilogue
  is ACT copy (+bias) then Pool add (+residual); host transposes
  [D, T] -> [T, D] after gather.
All weights stay SBUF-resident for the whole kernel (no restreaming);
phase A (LN) of block tb+1 is software-pipelined under the matmuls of
block tb, and the conv matmuls of block tb lag in_proj by 2 channel
blocks so PE never waits on the DVE gating chain. Half the out_proj
contraction chunks run as fp8 DoubleRow (OUT8_CHUNKS=8), the rest
bf16. PE runs ~92% busy at the mixed fp8/bf16 roofline; measured
~675 us on 8 cores (1.73x over the prior f32r implementation) at
rel err 1.67e-2.
"""

import sys
import types

import numpy as np

B, T, D, H = 8, 4096, 1024, 2048
KCONV = 4
N_CORES = 8
TB = 512            # tokens per t-block
NTB = T // TB       # 8
NSUB = TB // 128    # 4
KC = H // 128       # 16 channel blocks
DC = D // 128       # 8 d-chunks
EPS = 1e-5

# Which of the three in_proj thirds (gate_b, gate_c, hidden) use fp8
# DoubleRow; the rest use bf16 streamed weights.
FP8_THIRD = (True, True, True)
# Whether out_proj uses fp8 DoubleRow (hid2 quantized to fp8).
OUT_FP8 = False
# Number of leading out_proj contraction chunks (of KC=16) done in fp8
# DoubleRow instead of bf16; even, 0 disables. hid2 for those channel
# blocks is written fp8 directly by the conv tail.
OUT8_CHUNKS = 8
# Use DoubleRowSwInterleave (weights pre-interleaved on host in the
# PE's native load order -> contiguous LDWEIGHTS) for fp8 matmuls.
SWI = False
# Whether the depthwise conv matmuls use fp8 DoubleRow (gated hidden
# quantized to fp8; taps paired two per pass).
CONV_FP8 = False
WPAD = 528          # fp8 sub-row stride for paired conv taps (mult of 16)
# Channel blocks whose conv runs as diag-matmuls on PE; the rest run as
# a scalar_tensor_tensor FMA chain on DVE (balances the two engines).
CONV_PE_K = (14, 15)

_CACHE = {}


def _install_ntff_hook():
    if "antenv.axon_hooks" in sys.modules:
        return
    try:
        import trn_agent_boot.trn_boot as tb

        hook = tb._ntff_profile_via_ctypes("/opt/axon/libaxon_pjrt.so")
        mod = types.ModuleType("antenv.axon_hooks")
        mod.get_axon_ntff_profile_hook = lambda: hook
        mod.set_axon_ntff_profile_hook = lambda h: None
        sys.modules["antenv.axon_hooks"] = mod
    except Exception:
        pass


def _build_nc(ntb=NTB):
    import concourse.bass as bass  # noqa: F401
    import concourse.tile as tile
    from concourse import bacc, mybir

    f32 = mybir.dt.float32
    f32r = mybir.dt.float32r
    bf16 = mybir.dt.bfloat16
    fp8 = mybir.dt.float8e4
    AF = mybir.ActivationFunctionType
    OP = mybir.AluOpType
    DR = (mybir.MatmulPerfMode.DoubleRowSwInterleave if SWI
          else mybir.MatmulPerfMode.DoubleRow)

    n8 = sum(FP8_THIRD)          # fp8 thirds
    nb = 3 - n8                  # bf16 thirds
    tau8 = [t for t in range(3) if FP8_THIRD[t]]
    taub = [t for t in range(3) if not FP8_THIRD[t]]

    nc = bacc.Bacc("TRN2", target_bir_lowering=False, debug=False,
                   num_devices=N_CORES)
    x_d = nc.dram_tensor("xb16", [T, D], bf16, kind="ExternalInput").ap()
    xt_d = nc.dram_tensor("xt_proc", [NTB, 128, DC, TB], bf16,
                          kind="ExternalInput").ap()
    if n8:
        w8_d = nc.dram_tensor("w8_proc", [128, KC, n8, DC // 2, 2, 128],
                              fp8, kind="ExternalInput").ap()
    if nb:
        wb_d = nc.dram_tensor("wb_proc", [128, KC, nb, DC, 128], bf16,
                              kind="ExternalInput").ap()
    d_d = nc.dram_tensor("d_proc", [128, KC, 3], f32,
                         kind="ExternalInput").ap()
    cwdt = fp8 if CONV_FP8 else bf16
    NPE = len(CONV_PE_K)
    cw_d = nc.dram_tensor("cw_proc", [128, KC, KCONV], f32,
                          kind="ExternalInput").ap()
    cb_d = nc.dram_tensor("cb_proc", [128, KC], f32,
                          kind="ExternalInput").ap()
    NQ8 = 0 if OUT_FP8 else OUT8_CHUNKS
    if OUT_FP8:
        wout_d = nc.dram_tensor("wout_proc", [128, KC // 2, 2, DC, 128],
                                fp8, kind="ExternalInput").ap()
    else:
        wout_d = nc.dram_tensor("wout_proc", [128, KC - NQ8, DC, 128],
                                bf16, kind="ExternalInput").ap()
        if NQ8:
            wout8_d = nc.dram_tensor("wout8_proc",
                                     [128, NQ8 // 2, 2, DC, 128],
                                     fp8, kind="ExternalInput").ap()
    boutp_d = nc.dram_tensor("boutp", [128, DC], f32,
                             kind="ExternalInput").ap()
    iden_d = nc.dram_tensor("iden", [128, 128], f32,
                            kind="ExternalInput").ap()
    out_d = nc.dram_tensor("outT", [DC, 128, T], f32,
                           kind="ExternalOutput").ap()

    from contextlib import ExitStack

    with tile.TileContext(nc) as tc, ExitStack() as ctx:
        consts = ctx.enter_context(tc.tile_pool(name="consts", bufs=1))
        xtp = ctx.enter_context(tc.tile_pool(name="xtp", bufs=2))
        x8p = ctx.enter_context(tc.tile_pool(name="x8p", bufs=2))
        if nb:
            xbp = ctx.enter_context(tc.tile_pool(name="xbp", bufs=2))
            wbp = ctx.enter_context(tc.tile_pool(name="wbp", bufs=4))
        xp = ctx.enter_context(tc.tile_pool(name="xp", bufs=2))
        lnp = ctx.enter_context(tc.tile_pool(name="ln", bufs=4))
        rowp = ctx.enter_context(tc.tile_pool(name="rowp", bufs=2))
        rsp = ctx.enter_context(tc.tile_pool(name="rsp", bufs=2))
        scp = ctx.enter_context(tc.tile_pool(name="scp", bufs=2))
        hid2p = ctx.enter_context(tc.tile_pool(name="hid2", bufs=2))
        sgp = ctx.enter_context(tc.tile_pool(name="sg", bufs=4))
        hbp = ctx.enter_context(tc.tile_pool(name="hb", bufs=4))
        accp = ctx.enter_context(tc.tile_pool(name="acc", bufs=2))
        outp = ctx.enter_context(tc.tile_pool(name="outp", bufs=2))
        pb = ctx.enter_context(tc.tile_pool(name="pb", bufs=6, space="PSUM"))
        pc = ctx.enter_context(tc.tile_pool(name="pc", bufs=2, space="PSUM"))

        # ---- resident constants (tiny ones first; big ones split and
        #      interleaved with phase A of the first block) ----
        d_sb = consts.tile([128, KC, 3], f32)
        nc.sync.dma_start(out=d_sb, in_=d_d)
        cw_sb = consts.tile([128, KC, KCONV], f32)
        nc.sync.dma_start(out=cw_sb, in_=cw_d)
        cb_sb = consts.tile([128, KC], f32)
        nc.sync.dma_start(out=cb_sb, in_=cb_d)
        boutp_sb = consts.tile([128, DC], f32)
        nc.sync.dma_start(out=boutp_sb, in_=boutp_d)
        iden_sb = consts.tile([128, 128], f32)
        nc.sync.dma_start(out=iden_sb, in_=iden_d)
        ones_sb = consts.tile([1, 128], bf16)
        nc.vector.memset(ones_sb, 1.0)
        hbdt = fp8 if CONV_FP8 else bf16
        state = consts.tile([128, KC, 3], hbdt)
        nc.vector.memset(state, 0.0)
        w8_sb = None
        if n8:
            w8_sb = consts.tile([128, KC, n8, DC // 2, 2, 128], fp8)
        if NPE:
            cwd_sb = consts.tile([128, NPE, KCONV, 128], cwdt)
            cwd_built = set()
        if OUT_FP8:
            wout_sb = consts.tile([128, KC // 2, 2, DC, 128], fp8)
        else:
            wout_sb = consts.tile([128, KC - NQ8, DC, 128], bf16)
            if NQ8:
                wout8_sb = consts.tile([128, NQ8 // 2, 2, DC, 128], fp8)

        # per-tb rotating tiles, kept across pipeline stages
        cur = {}

        def phaseA_stats(tb):
            """DMA xT + natural x, bn stats -> mv4 [128, 2*NSUB]."""
            t0 = tb * TB
            xT = xtp.tile([128, DC, TB], bf16, name="xT")
            nc.sync.dma_start(out=xT, in_=xt_d[tb])
            mv4 = lnp.tile([128, 2 * NSUB], f32, name="mv4")
            for s in range(NSUB):
                xt = xp.tile([128, D], bf16, name="xt")
                nc.sync.dma_start(
                    out=xt, in_=x_d[t0 + s * 128:t0 + (s + 1) * 128, :])
                stats = lnp.tile([128, 2, 6], f32, name="stats")
                eng = nc.vector
                for g in range(2):
                    eng.bn_stats(
                        out=stats[:, g, :], in_=xt[:, g * 512:(g + 1) * 512])
                # deinterleaved layout: col [s] = mean, [NSUB+s] = var
                eng.bn_aggr(out=mv4[:, s::NSUB], in_=stats)
            cur[("xT", tb)] = xT
            cur[("mv4", tb)] = mv4

        def phaseA_transpose(tb):
            mv4 = cur.pop(("mv4", tb))
            pt = pb.tile([128, TB], f32, name="pbt")
            nc.tensor.transpose(pt[0:8, 0:128], mv4, iden_sb)
            mv8 = rowp.tile([8, 128], f32, name="mv8")
            nc.scalar.copy(out=mv8, in_=pt[0:8, 0:128])
            mvb = rowp.tile([8, 128], bf16, name="mvb")
            nc.scalar.copy(out=mvb, in_=pt[0:8, 0:128])
            # Newton-iterate y -> 1/sqrt(var+eps) on rows 4..7 (DVE only:
            # no Sqrt on ACT => act-func table never switches off sigmoid).
            vp = lnp.tile([8, 128], f32, name="vp")
            yt = lnp.tile([8, 128], f32, name="yt")
            at = lnp.tile([8, 128], f32, name="at")
            v, y, a = vp, yt, at
            nc.vector.tensor_scalar_add(out=v, in0=mv8, scalar1=EPS)
            nc.vector.tensor_scalar(out=y, in0=v, scalar1=-0.5,
                                    scalar2=1.5, op0=OP.mult, op1=OP.add)
            for _ in range(3):
                nc.vector.tensor_mul(out=a, in0=y, in1=y)
                nc.vector.tensor_mul(out=a, in0=a, in1=v)
                nc.vector.tensor_scalar(out=a, in0=a, scalar1=-0.5,
                                        scalar2=1.5, op0=OP.mult, op1=OP.add)
                nc.vector.tensor_mul(out=y, in0=y, in1=a)
            mrow = rowp.tile([1, NSUB, 128], bf16, name="mrow")
            nc.scalar.dma_start(out=mrow, in_=mvb[0:NSUB, :])
            ytb = lnp.tile([8, 128], bf16, name="ytb")
            nc.vector.tensor_copy(out=ytb, in_=yt)
            rrow = rowp.tile([1, NSUB, 128], bf16, name="rrow")
            nc.scalar.dma_start(out=rrow, in_=ytb[NSUB:2 * NSUB, :])
            cur[("mrow", tb)] = mrow
            cur[("rrow", tb)] = rrow

        def phaseA_broadcast(tb):
            mrow = cur.pop(("mrow", tb))
            rrow = cur.pop(("rrow", tb))
            mu_b = pc.tile([128, TB], f32, name="pct")
            nc.tensor.matmul(mu_b, ones_sb, mrow, start=True, stop=True)
            rs_b = pc.tile([128, TB], f32, name="pct")
            nc.tensor.matmul(rs_b, ones_sb, rrow, start=True, stop=True)
            rstd_b = rsp.tile([128, TB], f32, name="rstd_b")
            nc.scalar.activation(out=rstd_b, in_=rs_b, func=AF.Identity,
                                 bias=0.0, scale=1.0)
            mu_s = rsp.tile([128, TB], f32, name="mu_s")
            nc.scalar.activation(out=mu_s, in_=mu_b, func=AF.Identity,
                                 bias=0.0, scale=1.0)
            cur[("mu_s", tb)] = mu_s
            cur[("rstd_b", tb)] = rstd_b

        def phaseA_normalize(tb):
            mu_s = cur.pop(("mu_s", tb))
            rstd_b = cur.pop(("rstd_b", tb))
            xT = cur[("xT", tb)]
            xn8 = None
            xnb = None
            if n8:
                xn8 = x8p.tile([128, DC, TB], fp8, name="xn8")
            if nb:
                xnb = xbp.tile([128, DC, TB], bf16, name="xnb")
            for j in range(DC):
                eng = nc.vector if j % 2 == 0 else nc.gpsimd
                sc = scp.tile([128, TB], bf16, name="sc")
                eng.tensor_sub(out=sc, in0=xT[:, j, :], in1=mu_s)
                if n8:
                    eng.tensor_mul(out=xn8[:, j, :], in0=sc, in1=rstd_b)
                if nb:
                    eng.tensor_mul(out=xnb[:, j, :], in0=sc, in1=rstd_b)
            cur[("xn8", tb)] = xn8
            cur[("xnb", tb)] = xnb

        def phaseB_k(tb, k, hid2):
            xn8 = cur.get(("xn8", tb))
            xnb = cur.get(("xnb", tb))
            ps = [None] * 3
            for t8, tau in enumerate(tau8):
                p = pb.tile([128, TB], f32, name="pbt")
                for jj in range(DC // 2):
                    nc.tensor.matmul(
                        p, w8_sb[:, k, t8, jj],
                        xn8[:, 2 * jj:2 * jj + 2, :],
                        start=(jj == 0), stop=(jj == DC // 2 - 1),
                        perf_mode=DR)
                ps[tau] = p
            if nb:
                wgt = cur.pop(("wb", tb, k))
                for tb_i, tau in enumerate(taub):
                    p = pb.tile([128, TB], f32, name="pbt")
                    for j in range(DC):
                        nc.tensor.matmul(
                            p, wgt[:, tb_i, j], xnb[:, j, :],
                            start=(j == 0), stop=(j == DC - 1))
                    ps[tau] = p
            sigb = sgp.tile([128, TB], bf16, name="sigb")
            nc.scalar.activation(out=sigb, in_=ps[0], func=AF.Sigmoid,
                                 bias=d_sb[:, k, 0:1], scale=1.0)
            sigc = sgp.tile([128, TB], bf16, name="sigc")
            nc.scalar.activation(out=sigc, in_=ps[1], func=AF.Sigmoid,
                                 bias=d_sb[:, k, 1:2], scale=1.0)
            if CONV_FP8:
                hidb = hbp.tile([128, 2, WPAD], fp8, name="hidb")
                g = hidb[:, 0, :]
            else:
                hidb = hbp.tile([128, TB + 3], bf16, name="hidb")
                g = hidb
            nc.gpsimd.tensor_copy(out=g[:, 0:3], in_=state[:, k, :])
            nc.vector.scalar_tensor_tensor(
                out=g[:, 3:TB + 3], in0=ps[2],
                scalar=d_sb[:, k, 2:3], in1=sigb,
                op0=OP.add, op1=OP.mult)
            nc.gpsimd.tensor_copy(out=state[:, k, :],
                                  in_=g[:, TB:TB + 3])
            if CONV_FP8:
                # shifted duplicate for DoubleRow tap pairing
                nc.gpsimd.tensor_copy(out=hidb[:, 1, 0:TB + 2],
                                      in_=hidb[:, 0, 1:TB + 3])
            cur[("hidb", tb, k)] = hidb
            cur[("sigc", tb, k)] = sigc

        def conv_k(tb, k, hid2):
            """Depthwise conv as 4 PSUM-accumulated diag matmuls on PE."""
            hidb = cur.pop(("hidb", tb, k))
            sigc = cur.pop(("sigc", tb, k))
            ci0 = CONV_PE_K.index(k) if k in CONV_PE_K else None
            if ci0 is not None and k not in cwd_built:
                cwd_built.add(k)
                for kp in range(KCONV):
                    nc.scalar.activation(
                        out=cwd_sb[:, ci0, kp], in_=iden_sb,
                        func=AF.Identity, bias=0.0,
                        scale=cw_sb[:, k, kp:kp + 1])
            hid28 = cur.get("hid28")
            if hid28 is not None:
                h2dst = hid28[:, k, :] if k < NQ8 else hid2[:, k - NQ8, :]
            else:
                h2dst = hid2[:, k, :]
            if k in CONV_PE_K:
                ci = CONV_PE_K.index(k)
                psC = pb.tile([128, TB], f32, name="pbt")
                if CONV_FP8:
                    for pr in range(KCONV // 2):
                        nc.tensor.matmul(
                            psC, cwd_sb[:, ci, 2 * pr:2 * pr + 2],
                            hidb[:, :, 2 * pr:2 * pr + TB],
                            start=(pr == 0), stop=(pr == KCONV // 2 - 1),
                            perf_mode=DR)
                else:
                    for kp in range(KCONV):
                        nc.tensor.matmul(psC, cwd_sb[:, ci, kp],
                                         hidb[:, kp:kp + TB],
                                         start=(kp == 0),
                                         stop=(kp == KCONV - 1))
                nc.vector.scalar_tensor_tensor(
                    out=h2dst, in0=psC, scalar=cb_sb[:, k:k + 1],
                    in1=sigc, op0=OP.add, op1=OP.mult)
            else:
                a0 = accp.tile([128, TB], f32, name="acc0")
                a1 = accp.tile([128, TB], f32, name="acc1")
                nc.vector.tensor_scalar_mul(
                    out=a0, in0=hidb[:, 0:TB], scalar1=cw_sb[:, k, 0:1])
                nc.vector.scalar_tensor_tensor(
                    out=a1, in0=hidb[:, 1:TB + 1],
                    scalar=cw_sb[:, k, 1:2], in1=a0,
                    op0=OP.mult, op1=OP.add)
                nc.vector.scalar_tensor_tensor(
                    out=a0, in0=hidb[:, 2:TB + 2],
                    scalar=cw_sb[:, k, 2:3], in1=a1,
                    op0=OP.mult, op1=OP.add)
                nc.vector.scalar_tensor_tensor(
                    out=a1, in0=hidb[:, 3:TB + 3],
                    scalar=cw_sb[:, k, 3:4], in1=a0,
                    op0=OP.mult, op1=OP.add)
                nc.vector.scalar_tensor_tensor(
                    out=h2dst, in0=a1, scalar=cb_sb[:, k:k + 1],
                    in1=sigc, op0=OP.add, op1=OP.mult)

        def stream_wb(tb, k):
            if not nb:
                return
            wgt = wbp.tile([128, nb, DC, 128], bf16, name="wgt")
            nc.sync.dma_start(out=wgt, in_=wb_d[:, k])
            cur[("wb", tb, k)] = wgt

        def phaseC(tb, hid2):
            t0 = tb * TB
            xT = cur.pop(("xT", tb))
            for j in range(DC):
                p = pc.tile([128, TB], f32, name="pct")
                if OUT_FP8:
                    for kk in range(KC // 2):
                        nc.tensor.matmul(
                            p, wout_sb[:, kk, :, j],
                            hid2[:, 2 * kk:2 * kk + 2, :],
                            start=(kk == 0), stop=(kk == KC // 2 - 1),
                            perf_mode=DR)
                else:
                    hid28 = cur.get("hid28")
                    for kk in range(NQ8 // 2):
                        nc.tensor.matmul(
                            p, wout8_sb[:, kk, :, j],
                            hid28[:, 2 * kk:2 * kk + 2, :],
                            start=(kk == 0), stop=False, perf_mode=DR)
                    for k in range(KC - NQ8):
                        nc.tensor.matmul(
                            p, wout_sb[:, k, j], hid2[:, k, :],
                            start=(NQ8 == 0 and k == 0),
                            stop=(k == KC - NQ8 - 1))
                ot = outp.tile([128, TB], f32, name="ot")
                nc.scalar.activation(out=ot, in_=p, func=AF.Identity,
                                     bias=boutp_sb[:, j:j + 1], scale=1.0)
                nc.gpsimd.tensor_add(out=ot, in0=ot, in1=xT[:, j, :])
                dq = nc.scalar if (tb == ntb - 1 and j % 2) else nc.sync
                dq.dma_start(out=out_d[j, :, t0:t0 + TB], in_=ot)

        # ---- software-pipelined main loop ----
        phaseA_stats(0)
        if n8:
            for q in range(8):
                nc.sync.dma_start(out=w8_sb[:, 2 * q:2 * q + 2],
                                  in_=w8_d[:, 2 * q:2 * q + 2])
        if not OUT_FP8 and NQ8:
            nc.sync.dma_start(out=wout8_sb, in_=wout8_d)
        phaseA_transpose(0)
        phaseA_broadcast(0)
        phaseA_normalize(0)
        h2dt = fp8 if OUT_FP8 else bf16
        for tb in range(ntb):
            if OUT_FP8 or not NQ8:
                hid2 = hid2p.tile([128, KC, TB], h2dt, name="hid2")
                hid28 = None
            else:
                hid2 = hid2p.tile([128, KC - NQ8, TB], bf16, name="hid2")
                hid28 = hid2p.tile([128, NQ8, TB], fp8, name="hid28")
            cur["hid28"] = hid28
            nxt = tb + 1
            LAG = 2
            for k in range(KC):
                if nb and k == 0:
                    stream_wb(tb, 0)
                if nb and k < KC - 1:
                    stream_wb(tb, k + 1)
                phaseB_k(tb, k, hid2)
                if tb == 0 and k < 4:
                    q = k
                    nw = (KC - NQ8 if not OUT_FP8 else KC) // 4
                    lo, hi = nw * q, nw * (q + 1)
                    if q == 3:
                        hi = wout_sb.shape[1]
                    nc.sync.dma_start(out=wout_sb[:, lo:hi],
                                      in_=wout_d[:, lo:hi])
                if k >= LAG:
                    conv_k(tb, k - LAG, hid2)
                if nxt < ntb:
                    ks, ktr, kbc = (6, 9, 10) if tb == 0 else (3, 6, 7)
                    if k == ks:
                        phaseA_stats(nxt)
                    elif k == ktr:
                        phaseA_transpose(nxt)
                    elif k == kbc:
                        phaseA_broadcast(nxt)
                    elif k == KC - 1:
                        phaseA_normalize(nxt)
            for k in range(KC - LAG, KC):
                conv_k(tb, k, hid2)
            cur.pop(("xn8", tb), None)
            cur.pop(("xnb", tb), None)
            phaseC(tb, hid2)
    nc.compile()
    return nc


def _prep_inputs(x, ln_g, ln_b, w_in, b_in, conv_w, conv_b, w_out, b_out,
                 ntb=NTB):
    import ml_dtypes

    f8 = ml_dtypes.float8_e4m3
    x = np.asarray(x, np.float32)
    ln_g = np.asarray(ln_g, np.float32)
    ln_b = np.asarray(ln_b, np.float32)
    w_in = np.asarray(w_in, np.float32)
    b_in = np.asarray(b_in, np.float32)
    conv_w = np.asarray(conv_w, np.float32)
    conv_b = np.asarray(conv_b, np.float32)
    w_out = np.asarray(w_out, np.float32)
    b_out = np.asarray(b_out, np.float32)

    tau8 = [t for t in range(3) if FP8_THIRD[t]]
    taub = [t for t in range(3) if not FP8_THIRD[t]]

    def _swi(arr):
        # arr[..., 2, 128] pair-blocks -> SwInterleave layout:
        # flat[f] = W[f%2, 127 - f//2], reshaped back to [2, 128]
        if not SWI:
            return arr
        f = np.arange(256)
        out = arr[..., f % 2, 127 - f // 2]
        return out.reshape(arr.shape[:-2] + (2, 128))

    wg = w_in * ln_g[None, :]                       # [3H, D]
    dv = b_in + w_in @ ln_b                         # [3H]
    # [tau, k, m, jj, i, p] -> [p, k, tau, jj, i, m]
    wblk = wg.reshape(3, KC, 128, DC // 2, 2, 128)
    wperm = wblk.transpose(5, 1, 0, 3, 4, 2)        # [p, k, tau, jj, i, m]
    shared = {}
    if tau8:
        shared["w8_proc"] = np.ascontiguousarray(
            _swi(wperm[:, :, tau8])).astype(f8)
    if taub:
        wb = wg.reshape(3, KC, 128, DC, 128).transpose(4, 1, 0, 3, 2)
        shared["wb_proc"] = np.ascontiguousarray(
            wb[:, :, taub]).astype(ml_dtypes.bfloat16)
    shared["d_proc"] = np.ascontiguousarray(
        dv.reshape(3, KC, 128).transpose(2, 1, 0))
    cw = conv_w.reshape(KC, 128, KCONV)          # [k, m, kp]
    shared["cw_proc"] = np.ascontiguousarray(cw.transpose(1, 0, 2))
    shared["cb_proc"] = np.ascontiguousarray(conv_b.reshape(KC, 128).T)
    # wout_proc[p, k, j, m] = w_out[128j+m, 128k+p]
    wo = w_out.reshape(DC, 128, KC, 128).transpose(3, 2, 0, 1)
    if OUT_FP8:
        shared["wout_proc"] = np.ascontiguousarray(
            wo.reshape(128, KC // 2, 2, DC, 128)).astype(f8)
    else:
        nq8 = OUT8_CHUNKS
        shared["wout_proc"] = np.ascontiguousarray(
            wo[:, nq8:]).astype(ml_dtypes.bfloat16)
        if nq8:
            w8o = wo[:, :nq8].reshape(
                128, nq8 // 2, 2, DC, 128).transpose(0, 1, 3, 2, 4)
            w8o = _swi(w8o).transpose(0, 1, 3, 2, 4)
            shared["wout8_proc"] = np.ascontiguousarray(w8o).astype(f8)
    shared["boutp"] = np.ascontiguousarray(b_out.reshape(DC, 128).T)
    shared["iden"] = np.eye(128, dtype=np.float32)

    maps = []
    for b in range(B):
        xb = np.ascontiguousarray(x[b])
        # xt_proc[tb, p, j, t] = x[b, 512*tb + t, 128*j + p]
        xt_proc = np.ascontiguousarray(
            xb.T.reshape(DC, 128, NTB, TB).transpose(2, 1, 0, 3)).astype(
                ml_dtypes.bfloat16)
        maps.append(dict(shared, xb16=xb.astype(ml_dtypes.bfloat16),
                         xt_proc=xt_proc))
    return maps


def run(inputs, trace=False):
    _install_ntff_hook()
    import concourse.bass_utils as bu

    bu.upload_artifacts = lambda d: "local://" + d
    if "nc" not in _CACHE:
        _CACHE["nc"] = _build_nc()
    nc = _CACHE["nc"]
    in_maps = _prep_inputs(**inputs)
    res = bu.run_bass_kernel_spmd(nc, in_maps, list(range(N_CORES)),
                                  trace=trace)
    out = np.stack([
        np.ascontiguousarray(
            res.results[b]["outT"].reshape(D, T).T)
        for b in range(B)
    ])
    return out.astype(np.float32), res


def kernel(**inputs):
    out, _ = run(inputs, trace=False)
    return out


# revision 48
# speedup vs baseline: 1.7878x; 1.0124x over previous
"""Trainium2 Bass kernel for nn_GatedShortConvBlock.

Full inputs -> full output. Data-parallel over batch: batch b -> core b.
Per-core dataflow (T=4096 tokens, D=1024, H=2048, K=4), per 512-token
block:
  LN stats via bn_stats on DVE over natural-layout x; 1/sqrt(var+eps)
  via a 4-step Newton iteration on DVE (keeps ACT pinned to the sigmoid
  act-function table all kernel); mean/rstd rank-1-broadcast by PE and
  evacuated to SBUF by ACT; normalize+fp8-quantize on Pool.
  in_proj: fp8e4 DoubleRow matmuls (256-deep contraction per pass,
  2x bf16 rate) against SBUF-resident fp8 weights (LayerNorm gamma/beta
  folded into weights/bias on the host). Gates: sigmoid on ACT with
  fused per-channel bias, bf16 out. Gated hidden via one stt on DVE.
  Depthwise causal conv: 4 PSUM-accumulated PE matmuls per channel
  block with diagonal bf16 stationaries against free-dim-shifted
  windows of the gated signal; (+conv bias) * sigmoid gate fused in one
  DVE stt. out_proj: bf16 matmuls into [d, t]-layout output; epilogue
  is ACT copy (+bias) then Pool add (+residual); host transposes
  [D, T] -> [T, D] after gather.
All weights stay SBUF-resident for the whole kernel (no restreaming);
phase A (LN) of block tb+1 is software-pipelined under the matmuls of
block tb, and the conv matmuls of block tb lag in_proj by 2 channel
blocks so PE never waits on the DVE gating chain. Half the out_proj
contraction chunks run as fp8 DoubleRow (OUT8_CHUNKS=8), the rest
bf16. PE runs ~92% busy at the mixed fp8/bf16 roofline; measured
~675 us on 8 cores (1.73x over the prior f32r implementation) at
rel err 1.67e-2.
"""

import sys
import types

import numpy as np

B, T, D, H = 8, 4096, 1024, 2048
KCONV = 4
N_CORES = 8
TB = 512            # tokens per t-block
NTB = T // TB       # 8
NSUB = TB // 128    # 4
KC = H // 128       # 16 channel blocks
DC = D // 128       # 8 d-chunks
EPS = 1e-5

# Which of the three in_proj thirds (gate_b, gate_c, hidden) use fp8
# DoubleRow; the rest use bf16 streamed weights.
FP8_THIRD = (True, True, True)
# Whether out_proj uses fp8 DoubleRow (hid2 quantized to fp8).
OUT_FP8 = False
# Number of leading out_proj contraction chunks (of KC=16) done in fp8
# DoubleRow instead of bf16; even, 0 disables. hid2 for those channel
# blocks is written fp8 directly by the conv tail.
OUT8_CHUNKS = 8
# Use DoubleRowSwInterleave (weights pre-interleaved on host in the
# PE's native load order -> contiguous LDWEIGHTS) for fp8 matmuls.
SWI = False
# Whether the depthwise conv matmuls use fp8 DoubleRow (gated hidden
# quantized to fp8; taps paired two per pass).
CONV_FP8 = False
WPAD = 528          # fp8 sub-row stride for paired conv taps (mult of 16)
# Channel blocks whose conv runs as diag-matmuls on PE; the rest run as
# a scalar_tensor_tensor FMA chain on DVE (balances the two engines).
CONV_PE_K = (14, 15)

_CACHE = {}


def _install_ntff_hook():
    if "antenv.axon_hooks" in sys.modules:
        return
    try:
        import trn_agent_boot.trn_boot as tb

        hook = tb._ntff_profile_via_ctypes("/opt/axon/libaxon_pjrt.so")
        mod = types.ModuleType("antenv.axon_hooks")
        mod.get_axon_ntff_profile_hook = lambda: hook
        mod.set_axon_ntff_profile_hook = lambda h: None
        sys.modules["antenv.axon_hooks"] = mod
    except Exception:
        pass


def _build_nc(ntb=NTB):
    import concourse.bass as bass  # noqa: F401
    import concourse.tile as tile
    from concourse import bacc, mybir

    f32 = mybir.dt.float32
    f32r = mybir.dt.float32r
    bf16 = mybir.dt.bfloat16
    fp8 = mybir.dt.float8e4
    AF = mybir.ActivationFunctionType
    OP = mybir.AluOpType
    DR = (mybir.MatmulPerfMode.DoubleRowSwInterleave if SWI
          else mybir.MatmulPerfMode.DoubleRow)

    n8 = sum(FP8_THIRD)          # fp8 thirds
    nb = 3 - n8                  # bf16 thirds
    tau8 = [t for t in range(3) if FP8_THIRD[t]]
    taub = [t for t in range(3) if not FP8_THIRD[t]]

    nc = bacc.Bacc("TRN2", target_bir_lowering=False, debug=False,
                   num_devices=N_CORES)
    x_d = nc.dram_tensor("xb16", [T, D], bf16, kind="ExternalInput").ap()
    xt_d = nc.dram_tensor("xt_proc", [NTB, 128, DC, TB], bf16,
                          kind="ExternalInput").ap()
    if n8:
        w8_d = nc.dram_tensor("w8_proc", [128, KC, n8, DC // 2, 2, 128],
                              fp8, kind="ExternalInput").ap()
    if nb:
        wb_d = nc.dram_tensor("wb_proc", [128, KC, nb, DC, 128], bf16,
                              kind="ExternalInput").ap()
    d_d = nc.dram_tensor("d_proc", [128, KC, 3], f32,
                         kind="ExternalInput").ap()
    cwdt = fp8 if CONV_FP8 else bf16
    NPE = len(CONV_PE_K)
    cw_d = nc.dram_tensor("cw_proc", [128, KC, KCONV], f32,
                          kind="ExternalInput").ap()
    cb_d = nc.dram_tensor("cb_proc", [128, KC], f32,
                          kind="ExternalInput").ap()
    NQ8 = 0 if OUT_FP8 else OUT8_CHUNKS
    if OUT_FP8:
        wout_d = nc.dram_tensor("wout_proc", [128, KC // 2, 2, DC, 128],
                                fp8, kind="ExternalInput").ap()
    else:
        wout_d = nc.dram_tensor("wout_proc", [128, KC - NQ8, DC, 128],
                                bf16, kind="ExternalInput").ap()
        if NQ8:
            wout8_d = nc.dram_tensor("wout8_proc",
                                     [128, NQ8 // 2, 2, DC, 128],
                                     fp8, kind="ExternalInput").ap()
    boutp_d = nc.dram_tensor("boutp", [128, DC], f32,
                             kind="ExternalInput").ap()
    iden_d = nc.dram_tensor("iden", [128, 128], f32,
                            kind="ExternalInput").ap()
    out_d = nc.dram_tensor("outT", [DC, 128, T], f32,
                           kind="ExternalOutput").ap()

    from contextlib import ExitStack

    with tile.TileContext(nc) as tc, ExitStack() as ctx:
        consts = ctx.enter_context(tc.tile_pool(name="consts", bufs=1))
        xtp = ctx.enter_context(tc.tile_pool(name="xtp", bufs=2))
        x8p = ctx.enter_context(tc.tile_pool(name="x8p", bufs=2))
        if nb:
            xbp = ctx.enter_context(tc.tile_pool(name="xbp", bufs=2))
            wbp = ctx.enter_context(tc.tile_pool(name="wbp", bufs=4))
        xp = ctx.enter_context(tc.tile_pool(name="xp", bufs=2))
        lnp = ctx.enter_context(tc.tile_pool(name="ln", bufs=4))
        rowp = ctx.enter_context(tc.tile_pool(name="rowp", bufs=2))
        rsp = ctx.enter_context(tc.tile_pool(name="rsp", bufs=2))
        scp = ctx.enter_context(tc.tile_pool(name="scp", bufs=2))
        hid2p = ctx.enter_context(tc.tile_pool(name="hid2", bufs=2))
        sgp = ctx.enter_context(tc.tile_pool(name="sg", bufs=4))
        hbp = ctx.enter_context(tc.tile_pool(name="hb", bufs=4))
        accp = ctx.enter_context(tc.tile_pool(name="acc", bufs=2))
        outp = ctx.enter_context(tc.tile_pool(name="outp", bufs=2))
        pb = ctx.enter_context(tc.tile_pool(name="pb", bufs=6, space="PSUM"))
        pc = ctx.enter_context(tc.tile_pool(name="pc", bufs=2, space="PSUM"))

        # ---- resident constants (tiny ones first; big ones split and
        #      interleaved with phase A of the first block) ----
        d_sb = consts.tile([128, KC, 3], f32)
        nc.sync.dma_start(out=d_sb, in_=d_d)
        cw_sb = consts.tile([128, KC, KCONV], f32)
        nc.sync.dma_start(out=cw_sb, in_=cw_d)
        cb_sb = consts.tile([128, KC], f32)
        nc.sync.dma_start(out=cb_sb, in_=cb_d)
        boutp_sb = consts.tile([128, DC], f32)
        nc.sync.dma_start(out=boutp_sb, in_=boutp_d)
        iden_sb = consts.tile([128, 128], f32)
        nc.sync.dma_start(out=iden_sb, in_=iden_d)
        ones_sb = consts.tile([1, 128], bf16)
        nc.vector.memset(ones_sb, 1.0)
        # dummy matmul on memset-only data: first PE instruction, no DMA
        # dependency -- absorbs cold sequencer/pipeline warmup and starts
        # the HAM clock ramp while weights stream in.
        pdum = pc.tile([128, TB], f32, name="pct")
        nc.tensor.matmul(pdum[:, 0:128], ones_sb, ones_sb,
                         start=True, stop=True)
        hbdt = fp8 if CONV_FP8 else bf16
        state = consts.tile([128, KC, 3], hbdt)
        nc.vector.memset(state, 0.0)
        w8_sb = None
        if n8:
            w8_sb = consts.tile([128, KC, n8, DC // 2, 2, 128], fp8)
        if NPE:
            cwd_sb = consts.tile([128, NPE, KCONV, 128], cwdt)
            cwd_built = set()
        if OUT_FP8:
            wout_sb = consts.tile([128, KC // 2, 2, DC, 128], fp8)
        else:
            wout_sb = consts.tile([128, KC - NQ8, DC, 128], bf16)
            if NQ8:
                wout8_sb = consts.tile([128, NQ8 // 2, 2, DC, 128], fp8)

        # per-tb rotating tiles, kept across pipeline stages
        cur = {}

        def phaseA_stats(tb):
            """DMA xT + natural x, bn stats -> mv4 [128, 2*NSUB]."""
            t0 = tb * TB
            xT = xtp.tile([128, DC, TB], bf16, name="xT")
            nc.sync.dma_start(out=xT, in_=xt_d[tb])
            mv4 = lnp.tile([128, 2 * NSUB], f32, name="mv4")
            for s in range(NSUB):
                xt = xp.tile([128, D], bf16, name="xt")
                nc.sync.dma_start(
                    out=xt, in_=x_d[t0 + s * 128:t0 + (s + 1) * 128, :])
                stats = lnp.tile([128, 2, 6], f32, name="stats")
                eng = nc.vector
                for g in range(2):
                    eng.bn_stats(
                        out=stats[:, g, :], in_=xt[:, g * 512:(g + 1) * 512])
                # deinterleaved layout: col [s] = mean, [NSUB+s] = var
                eng.bn_aggr(out=mv4[:, s::NSUB], in_=stats)
            cur[("xT", tb)] = xT
            cur[("mv4", tb)] = mv4

        def phaseA_transpose(tb):
            mv4 = cur.pop(("mv4", tb))
            pt = pb.tile([128, TB], f32, name="pbt")
            nc.tensor.transpose(pt[0:8, 0:128], mv4, iden_sb)
            mv8 = rowp.tile([8, 128], f32, name="mv8")
            nc.scalar.copy(out=mv8, in_=pt[0:8, 0:128])
            mvb = rowp.tile([8, 128], bf16, name="mvb")
            nc.scalar.copy(out=mvb, in_=pt[0:8, 0:128])
            # Newton-iterate y -> 1/sqrt(var+eps) on rows 4..7 (DVE only:
            # no Sqrt on ACT => act-func table never switches off sigmoid).
            vp = lnp.tile([8, 128], f32, name="vp")
            yt = lnp.tile([8, 128], f32, name="yt")
            at = lnp.tile([8, 128], f32, name="at")
            v, y, a = vp, yt, at
            nc.vector.tensor_scalar_add(out=v, in0=mv8, scalar1=EPS)
            nc.vector.tensor_scalar(out=y, in0=v, scalar1=-0.5,
                                    scalar2=1.5, op0=OP.mult, op1=OP.add)
            for _ in range(3):
                nc.vector.tensor_mul(out=a, in0=y, in1=y)
                nc.vector.tensor_mul(out=a, in0=a, in1=v)
                nc.vector.tensor_scalar(out=a, in0=a, scalar1=-0.5,
                                        scalar2=1.5, op0=OP.mult, op1=OP.add)
                nc.vector.tensor_mul(out=y, in0=y, in1=a)
            mrow = rowp.tile([1, NSUB, 128], bf16, name="mrow")
            nc.scalar.dma_start(out=mrow, in_=mvb[0:NSUB, :])
            ytb = lnp.tile([8, 128], bf16, name="ytb")
            nc.vector.tensor_copy(out=ytb, in_=yt)
            rrow = rowp.tile([1, NSUB, 128], bf16, name="rrow")
            nc.scalar.dma_start(out=rrow, in_=ytb[NSUB:2 * NSUB, :])
            cur[("mrow", tb)] = mrow
            cur[("rrow", tb)] = rrow

        def phaseA_broadcast(tb):
            mrow = cur.pop(("mrow", tb))
            rrow = cur.pop(("rrow", tb))
            mu_b = pc.tile([128, TB], f32, name="pct")
            nc.tensor.matmul(mu_b, ones_sb, mrow, start=True, stop=True)
            rs_b = pc.tile([128, TB], f32, name="pct")
            nc.tensor.matmul(rs_b, ones_sb, rrow, start=True, stop=True)
            rstd_b = rsp.tile([128, TB], f32, name="rstd_b")
            nc.scalar.activation(out=rstd_b, in_=rs_b, func=AF.Identity,
                                 bias=0.0, scale=1.0)
            mu_s = rsp.tile([128, TB], f32, name="mu_s")
            nc.scalar.activation(out=mu_s, in_=mu_b, func=AF.Identity,
                                 bias=0.0, scale=1.0)
            cur[("mu_s", tb)] = mu_s
            cur[("rstd_b", tb)] = rstd_b

        def phaseA_normalize(tb):
            mu_s = cur.pop(("mu_s", tb))
            rstd_b = cur.pop(("rstd_b", tb))
            xT = cur[("xT", tb)]
            xn8 = None
            xnb = None
            if n8:
                xn8 = x8p.tile([128, DC, TB], fp8, name="xn8")
            if nb:
                xnb = xbp.tile([128, DC, TB], bf16, name="xnb")
            for j in range(DC):
                eng = nc.vector if j % 2 == 0 else nc.gpsimd
                sc = scp.tile([128, TB], bf16, name="sc")
                eng.tensor_sub(out=sc, in0=xT[:, j, :], in1=mu_s)
                if n8:
                    eng.tensor_mul(out=xn8[:, j, :], in0=sc, in1=rstd_b)
                if nb:
                    eng.tensor_mul(out=xnb[:, j, :], in0=sc, in1=rstd_b)
            cur[("xn8", tb)] = xn8
            cur[("xnb", tb)] = xnb

        def phaseB_k(tb, k, hid2):
            xn8 = cur.get(("xn8", tb))
            xnb = cur.get(("xnb", tb))
            ps = [None] * 3
            for t8, tau in enumerate(tau8):
                p = pb.tile([128, TB], f32, name="pbt")
                for jj in range(DC // 2):
                    nc.tensor.matmul(
                        p, w8_sb[:, k, t8, jj],
                        xn8[:, 2 * jj:2 * jj + 2, :],
                        start=(jj == 0), stop=(jj == DC // 2 - 1),
                        perf_mode=DR)
                ps[tau] = p
            if nb:
                wgt = cur.pop(("wb", tb, k))
                for tb_i, tau in enumerate(taub):
                    p = pb.tile([128, TB], f32, name="pbt")
                    for j in range(DC):
                        nc.tensor.matmul(
                            p, wgt[:, tb_i, j], xnb[:, j, :],
                            start=(j == 0), stop=(j == DC - 1))
                    ps[tau] = p
            sigb = sgp.tile([128, TB], bf16, name="sigb")
            nc.scalar.activation(out=sigb, in_=ps[0], func=AF.Sigmoid,
                                 bias=d_sb[:, k, 0:1], scale=1.0)
            sigc = sgp.tile([128, TB], bf16, name="sigc")
            nc.scalar.activation(out=sigc, in_=ps[1], func=AF.Sigmoid,
                                 bias=d_sb[:, k, 1:2], scale=1.0)
            if CONV_FP8:
                hidb = hbp.tile([128, 2, WPAD], fp8, name="hidb")
                g = hidb[:, 0, :]
            else:
                hidb = hbp.tile([128, TB + 3], bf16, name="hidb")
                g = hidb
            nc.gpsimd.tensor_copy(out=g[:, 0:3], in_=state[:, k, :])
            nc.vector.scalar_tensor_tensor(
                out=g[:, 3:TB + 3], in0=ps[2],
                scalar=d_sb[:, k, 2:3], in1=sigb,
                op0=OP.add, op1=OP.mult)
            nc.gpsimd.tensor_copy(out=state[:, k, :],
                                  in_=g[:, TB:TB + 3])
            if CONV_FP8:
                # shifted duplicate for DoubleRow tap pairing
                nc.gpsimd.tensor_copy(out=hidb[:, 1, 0:TB + 2],
                                      in_=hidb[:, 0, 1:TB + 3])
            cur[("hidb", tb, k)] = hidb
            cur[("sigc", tb, k)] = sigc

        def conv_k(tb, k, hid2):
            """Depthwise conv as 4 PSUM-accumulated diag matmuls on PE."""
            hidb = cur.pop(("hidb", tb, k))
            sigc = cur.pop(("sigc", tb, k))
            ci0 = CONV_PE_K.index(k) if k in CONV_PE_K else None
            if ci0 is not None and k not in cwd_built:
                cwd_built.add(k)
                for kp in range(KCONV):
                    nc.scalar.activation(
                        out=cwd_sb[:, ci0, kp], in_=iden_sb,
                        func=AF.Identity, bias=0.0,
                        scale=cw_sb[:, k, kp:kp + 1])
            hid28 = cur.get("hid28")
            if hid28 is not None:
                h2dst = hid28[:, k, :] if k < NQ8 else hid2[:, k - NQ8, :]
            else:
                h2dst = hid2[:, k, :]
            if k in CONV_PE_K:
                ci = CONV_PE_K.index(k)
                psC = pb.tile([128, TB], f32, name="pbt")
                if CONV_FP8:
                    for pr in range(KCONV // 2):
                        nc.tensor.matmul(
                            psC, cwd_sb[:, ci, 2 * pr:2 * pr + 2],
                            hidb[:, :, 2 * pr:2 * pr + TB],
                            start=(pr == 0), stop=(pr == KCONV // 2 - 1),
                            perf_mode=DR)
                else:
                    for kp in range(KCONV):
                        nc.tensor.matmul(psC, cwd_sb[:, ci, kp],
                                         hidb[:, kp:kp + TB],
                                         start=(kp == 0),
                                         stop=(kp == KCONV - 1))
                nc.vector.scalar_tensor_tensor(
                    out=h2dst, in0=psC, scalar=cb_sb[:, k:k + 1],
                    in1=sigc, op0=OP.add, op1=OP.mult)
            else:
                a0 = accp.tile([128, TB], f32, name="acc0")
                a1 = accp.tile([128, TB], f32, name="acc1")
                nc.vector.tensor_scalar_mul(
                    out=a0, in0=hidb[:, 0:TB], scalar1=cw_sb[:, k, 0:1])
                nc.vector.scalar_tensor_tensor(
                    out=a1, in0=hidb[:, 1:TB + 1],
                    scalar=cw_sb[:, k, 1:2], in1=a0,
                    op0=OP.mult, op1=OP.add)
                nc.vector.scalar_tensor_tensor(
                    out=a0, in0=hidb[:, 2:TB + 2],
                    scalar=cw_sb[:, k, 2:3], in1=a1,
                    op0=OP.mult, op1=OP.add)
                nc.vector.scalar_tensor_tensor(
                    out=a1, in0=hidb[:, 3:TB + 3],
                    scalar=cw_sb[:, k, 3:4], in1=a0,
                    op0=OP.mult, op1=OP.add)
                nc.vector.scalar_tensor_tensor(
                    out=h2dst, in0=a1, scalar=cb_sb[:, k:k + 1],
                    in1=sigc, op0=OP.add, op1=OP.mult)

        def stream_wb(tb, k):
            if not nb:
                return
            wgt = wbp.tile([128, nb, DC, 128], bf16, name="wgt")
            nc.sync.dma_start(out=wgt, in_=wb_d[:, k])
            cur[("wb", tb, k)] = wgt

        def phaseC(tb, hid2):
            t0 = tb * TB
            xT = cur.pop(("xT", tb))
            for j in range(DC):
                p = pc.tile([128, TB], f32, name="pct")
                if OUT_FP8:
                    for kk in range(KC // 2):
                        nc.tensor.matmul(
                            p, wout_sb[:, kk, :, j],
                            hid2[:, 2 * kk:2 * kk + 2, :],
                            start=(kk == 0), stop=(kk == KC // 2 - 1),
                            perf_mode=DR)
                else:
                    hid28 = cur.get("hid28")
                    for kk in range(NQ8 // 2):
                        nc.tensor.matmul(
                            p, wout8_sb[:, kk, :, j],
                            hid28[:, 2 * kk:2 * kk + 2, :],
                            start=(kk == 0), stop=False, perf_mode=DR)
                    for k in range(KC - NQ8):
                        nc.tensor.matmul(
                            p, wout_sb[:, k, j], hid2[:, k, :],
                            start=(NQ8 == 0 and k == 0),
                            stop=(k == KC - NQ8 - 1))
                ot = outp.tile([128, TB], f32, name="ot")
                nc.scalar.activation(out=ot, in_=p, func=AF.Identity,
                                     bias=boutp_sb[:, j:j + 1], scale=1.0)
                nc.gpsimd.tensor_add(out=ot, in0=ot, in1=xT[:, j, :])
                dq = nc.scalar if (tb == ntb - 1 and j % 2) else nc.sync
                dq.dma_start(out=out_d[j, :, t0:t0 + TB], in_=ot)

        # ---- software-pipelined main loop ----
        phaseA_stats(0)
        if n8:
            for q in range(8):
                nc.sync.dma_start(out=w8_sb[:, 2 * q:2 * q + 2],
                                  in_=w8_d[:, 2 * q:2 * q + 2])
        if not OUT_FP8 and NQ8:
            nc.sync.dma_start(out=wout8_sb, in_=wout8_d)
        phaseA_transpose(0)
        phaseA_broadcast(0)
        phaseA_normalize(0)
        h2dt = fp8 if OUT_FP8 else bf16
        for tb in range(ntb):
            if OUT_FP8 or not NQ8:
                hid2 = hid2p.tile([128, KC, TB], h2dt, name="hid2")
                hid28 = None
            else:
                hid2 = hid2p.tile([128, KC - NQ8, TB], bf16, name="hid2")
                hid28 = hid2p.tile([128, NQ8, TB], fp8, name="hid28")
            cur["hid28"] = hid28
            nxt = tb + 1
            LAG = 2
            for k in range(KC):
                if nb and k == 0:
                    stream_wb(tb, 0)
                if nb and k < KC - 1:
                    stream_wb(tb, k + 1)
                phaseB_k(tb, k, hid2)
                if tb == 0 and k < 4:
                    q = k
                    nw = (KC - NQ8 if not OUT_FP8 else KC) // 4
                    lo, hi = nw * q, nw * (q + 1)
                    if q == 3:
                        hi = wout_sb.shape[1]
                    nc.sync.dma_start(out=wout_sb[:, lo:hi],
                                      in_=wout_d[:, lo:hi])
                if k >= LAG:
                    conv_k(tb, k - LAG, hid2)
                if nxt < ntb:
                    ks, ktr, kbc = (6, 9, 10) if tb == 0 else (3, 6, 7)
                    if k == ks:
                        phaseA_stats(nxt)
                    elif k == ktr:
                        phaseA_transpose(nxt)
                    elif k == kbc:
                        phaseA_broadcast(nxt)
                    elif k == KC - 1:
                        phaseA_normalize(nxt)
            for k in range(KC - LAG, KC):
                conv_k(tb, k, hid2)
            cur.pop(("xn8", tb), None)
            cur.pop(("xnb", tb), None)
            phaseC(tb, hid2)
    nc.compile()
    return nc


def _prep_inputs(x, ln_g, ln_b, w_in, b_in, conv_w, conv_b, w_out, b_out,
                 ntb=NTB):
    import ml_dtypes

    f8 = ml_dtypes.float8_e4m3
    x = np.asarray(x, np.float32)
    ln_g = np.asarray(ln_g, np.float32)
    ln_b = np.asarray(ln_b, np.float32)
    w_in = np.asarray(w_in, np.float32)
    b_in = np.asarray(b_in, np.float32)
    conv_w = np.asarray(conv_w, np.float32)
    conv_b = np.asarray(conv_b, np.float32)
    w_out = np.asarray(w_out, np.float32)
    b_out = np.asarray(b_out, np.float32)

    tau8 = [t for t in range(3) if FP8_THIRD[t]]
    taub = [t for t in range(3) if not FP8_THIRD[t]]

    def _swi(arr):
        # arr[..., 2, 128] pair-blocks -> SwInterleave layout:
        # flat[f] = W[f%2, 127 - f//2], reshaped back to [2, 128]
        if not SWI:
            return arr
        f = np.arange(256)
        out = arr[..., f % 2, 127 - f // 2]
        return out.reshape(arr.shape[:-2] + (2, 128))

    wg = w_in * ln_g[None, :]                       # [3H, D]
    dv = b_in + w_in @ ln_b                         # [3H]
    # [tau, k, m, jj, i, p] -> [p, k, tau, jj, i, m]
    wblk = wg.reshape(3, KC, 128, DC // 2, 2, 128)
    wperm = wblk.transpose(5, 1, 0, 3, 4, 2)        # [p, k, tau, jj, i, m]
    shared = {}
    if tau8:
        shared["w8_proc"] = np.ascontiguousarray(
            _swi(wperm[:, :, tau8])).astype(f8)
    if taub:
        wb = wg.reshape(3, KC, 128, DC, 128).transpose(4, 1, 0, 3, 2)
        shared["wb_proc"] = np.ascontiguousarray(
            wb[:, :, taub]).astype(ml_dtypes.bfloat16)
    shared["d_proc"] = np.ascontiguousarray(
        dv.reshape(3, KC, 128).transpose(2, 1, 0))
    cw = conv_w.reshape(KC, 128, KCONV)          # [k, m, kp]
    shared["cw_proc"] = np.ascontiguousarray(cw.transpose(1, 0, 2))
    shared["cb_proc"] = np.ascontiguousarray(conv_b.reshape(KC, 128).T)
    # wout_proc[p, k, j, m] = w_out[128j+m, 128k+p]
    wo = w_out.reshape(DC, 128, KC, 128).transpose(3, 2, 0, 1)
    if OUT_FP8:
        shared["wout_proc"] = np.ascontiguousarray(
            wo.reshape(128, KC // 2, 2, DC, 128)).astype(f8)
    else:
        nq8 = OUT8_CHUNKS
        shared["wout_proc"] = np.ascontiguousarray(
            wo[:, nq8:]).astype(ml_dtypes.bfloat16)
        if nq8:
            w8o = wo[:, :nq8].reshape(
                128, nq8 // 2, 2, DC, 128).transpose(0, 1, 3, 2, 4)
            w8o = _swi(w8o).transpose(0, 1, 3, 2, 4)
            shared["wout8_proc"] = np.ascontiguousarray(w8o).astype(f8)
    shared["boutp"] = np.ascontiguousarray(b_out.reshape(DC, 128).T)
    shared["iden"] = np.eye(128, dtype=np.float32)

    maps = []
    for b in range(B):
        xb = np.ascontiguousarray(x[b])
        # xt_proc[tb, p, j, t] = x[b, 512*tb + t, 128*j + p]
        xt_proc = np.ascontiguousarray(
            xb.T.reshape(DC, 128, NTB, TB).transpose(2, 1, 0, 3)).astype(
                ml_dtypes.bfloat16)
        maps.append(dict(shared, xb16=xb.astype(ml_dtypes.bfloat16),
                         xt_proc=xt_proc))
    return maps


def run(inputs, trace=False):
    _install_ntff_hook()
    import concourse.bass_utils as bu

    bu.upload_artifacts = lambda d: "local://" + d
    if "nc" not in _CACHE:
        _CACHE["nc"] = _build_nc()
    nc = _CACHE["nc"]
    in_maps = _prep_inputs(**inputs)
    res = bu.run_bass_kernel_spmd(nc, in_maps, list(range(N_CORES)),
                                  trace=trace)
    out = np.stack([
        np.ascontiguousarray(
            res.results[b]["outT"].reshape(D, T).T)
        for b in range(B)
    ])
    return out.astype(np.float32), res


def kernel(**inputs):
    out, _ = run(inputs, trace=False)
    return out
